# revision 1
# baseline (speedup 1.0000x reference)
"""MixGCF negative-sampling + BPR loss kernel for 8x Trainium2 NeuronCores.

Strategy (data-parallel over batch, tables replicated):
  - 8 cores x 256 users each (2 chunks of 128 users = partitions).
  - Per chunk: indirect-DMA gather of user/pos rows and all 128 candidate
    item rows (128 rows per call, 1KB rows), DVE mul+reduce for per-hop
    scores, argmax via max/iota-onehot trick, second small indirect gather
    of the selected rows, then the BPR loss reduced to per-partition
    partial sums. Host combines 8x[128,2] partials into 3 scalars.
"""
import sys

sys.path.insert(0, "/opt/trn_rl_repo")
import numpy as np

N_USERS = 200000
N_ITEMS = 200000
HOPS = 4
DIM = 64
BATCH = 2048
N_NEGS = 64
K = 2
DECAY = 1e-4
NCORES = 8
ROW = HOPS * DIM          # 256 f32 per table row
B_LOC = BATCH // NCORES   # 256 users per core
NCHUNK = B_LOC // 128     # 2 chunks of 128 users
NCAND = K * N_NEGS        # 128 candidates per user
SUB = 32                  # candidates gathered/scored per inner step
NSUB = NCAND // SUB       # 4

_CACHE = {}


def _build_bass(stage=99):
    import concourse.bass as bass
    import concourse.tile as tile
    from concourse import bacc, mybir

    f32 = mybir.dt.float32
    i32 = mybir.dt.int32
    Alu = mybir.AluOpType
    Act = mybir.ActivationFunctionType

    nc = bacc.Bacc("TRN2", target_bir_lowering=False, debug=False,
                   num_devices=NCORES)
    user_tab = nc.dram_tensor("user_tab", [N_USERS, ROW], f32,
                              kind="ExternalInput").ap()
    item_tab = nc.dram_tensor("item_tab", [N_ITEMS, ROW], f32,
                              kind="ExternalInput").ap()
    uid = nc.dram_tensor("uid", [128, NCHUNK], i32, kind="ExternalInput").ap()
    pid = nc.dram_tensor("pid", [128, NCHUNK], i32, kind="ExternalInput").ap()
    nid = nc.dram_tensor("nid", [128, NCHUNK, NCAND], i32,
                         kind="ExternalInput").ap()
    seed = nc.dram_tensor("seed", [128, NCHUNK], f32,
                          kind="ExternalInput").ap()
    part = nc.dram_tensor("part", [128, 2], f32, kind="ExternalOutput").ap()

    with tile.TileContext(nc) as tc:
        with tc.tile_pool(name="meta", bufs=1) as meta, \
             tc.tile_pool(name="gat", bufs=5) as gatp, \
             tc.tile_pool(name="sp", bufs=2) as spp, \
             tc.tile_pool(name="sn", bufs=2) as snp, \
             tc.tile_pool(name="sel", bufs=2) as selp, \
             tc.tile_pool(name="small", bufs=2) as smallp:
            # ---- static/meta staging ----
            uid_t = meta.tile([128, NCHUNK], i32)
            pid_t = meta.tile([128, NCHUNK], i32)
            nid_t = meta.tile([128, NCHUNK, NCAND], i32)
            seed_t = meta.tile([128, NCHUNK], f32)
            nc.sync.dma_start(uid_t[:], uid)
            nc.sync.dma_start(pid_t[:], pid)
            nc.sync.dma_start(nid_t[:], nid)
            nc.sync.dma_start(seed_t[:], seed)

            oms_t = meta.tile([128, NCHUNK], f32)   # 1 - seed
            nc.vector.tensor_scalar(oms_t[:], seed_t[:], -1.0, 1.0,
                                    Alu.mult, Alu.add)
            nidf_t = meta.tile([128, NCHUNK, NCAND], f32)
            nc.vector.tensor_copy(nidf_t[:], nid_t[:])

            iota_rev = meta.tile([128, N_NEGS], i32)  # 64 - n
            nc.gpsimd.iota(iota_rev[:], pattern=[[-1, N_NEGS]], base=N_NEGS,
                           channel_multiplier=0)
            iota_rev_f = meta.tile([128, N_NEGS], f32)
            nc.vector.tensor_copy(iota_rev_f[:], iota_rev[:])

            part_t = meta.tile([128, 2], f32)
            nc.vector.memset(part_t[:], 0.0)

            for ch in range(NCHUNK if stage >= 2 else 0):
                seed_ap = seed_t[:, ch:ch + 1]
                oms_ap = oms_t[:, ch:ch + 1]

                # ---- user/pos row gathers ----
                s_t = spp.tile([128, ROW], f32, tag="s")
                p_t = spp.tile([128, ROW], f32, tag="p")
                nc.gpsimd.indirect_dma_start(
                    out=s_t[:], out_offset=None, in_=user_tab,
                    in_offset=bass.IndirectOffsetOnAxis(
                        ap=uid_t[:, ch:ch + 1], axis=0))
                nc.gpsimd.indirect_dma_start(
                    out=p_t[:], out_offset=None, in_=item_tab,
                    in_offset=bass.IndirectOffsetOnAxis(
                        ap=pid_t[:, ch:ch + 1], axis=0))

                if stage <= 2:
                    continue
                # ---- candidate gathers + scoring ----
                sn_t = snp.tile([128, NCAND, HOPS], f32, tag="sn")
                for nt in range(NSUB):
                    gat = gatp.tile([128, SUB, ROW], f32, tag="gat")
                    for j in range(SUB):
                        col = nt * SUB + j
                        nc.gpsimd.indirect_dma_start(
                            out=gat[:, j], out_offset=None,
                            in_=item_tab,
                            in_offset=bass.IndirectOffsetOnAxis(
                                ap=nid_t[:, ch, col:col + 1], axis=0))
                    nc.vector.tensor_tensor(
                        out=gat[:], in0=gat[:],
                        in1=s_t[:].unsqueeze(1).to_broadcast([128, SUB, ROW]),
                        op=Alu.mult)
                    nc.vector.tensor_reduce(
                        out=sn_t[:, nt * SUB:(nt + 1) * SUB, :],
                        in_=gat[:].rearrange("p s (h d) -> p s h d", h=HOPS),
                        axis=mybir.AxisListType.X, op=Alu.add)

                if stage <= 3:
                    continue
                # ---- argmax per (k, hop) via onehot trick ----
                g_t = snp.tile([128, NCAND, HOPS], f32, tag="g")
                nc.vector.tensor_scalar_mul(g_t[:], sn_t[:], oms_ap)

                candf = smallp.tile([128, K * HOPS], f32, tag="candf")
                for k in range(K):
                    gk = g_t[:, k * N_NEGS:(k + 1) * N_NEGS, :].transpose(
                        [0, 2, 1])                     # [128, H, N]
                    m_k = smallp.tile([128, HOPS], f32, tag="mk")
                    nc.vector.tensor_reduce(out=m_k[:], in_=gk,
                                            axis=mybir.AxisListType.X,
                                            op=Alu.max)
                    eq = smallp.tile([128, HOPS, N_NEGS], f32, tag="eq")
                    nc.vector.tensor_tensor(
                        out=eq[:], in0=gk,
                        in1=m_k[:].unsqueeze(2).to_broadcast(
                            [128, HOPS, N_NEGS]),
                        op=Alu.is_equal)
                    w = smallp.tile([128, HOPS, N_NEGS], f32, tag="w")
                    nc.vector.tensor_tensor(
                        out=w[:], in0=eq[:],
                        in1=iota_rev_f[:].unsqueeze(1).to_broadcast(
                            [128, HOPS, N_NEGS]),
                        op=Alu.mult)
                    wmax = smallp.tile([128, HOPS], f32, tag="wmax")
                    nc.vector.tensor_reduce(out=wmax[:], in_=w[:],
                                            axis=mybir.AxisListType.X,
                                            op=Alu.max)
                    onehot = smallp.tile([128, HOPS, N_NEGS], f32, tag="oh")
                    nc.vector.tensor_tensor(
                        out=onehot[:],
                        in0=iota_rev_f[:].unsqueeze(1).to_broadcast(
                            [128, HOPS, N_NEGS]),
                        in1=wmax[:].unsqueeze(2).to_broadcast(
                            [128, HOPS, N_NEGS]),
                        op=Alu.is_equal)
                    idsel = smallp.tile([128, HOPS, N_NEGS], f32, tag="ids")
                    nc.vector.tensor_tensor(
                        out=idsel[:], in0=onehot[:],
                        in1=nidf_t[:, ch, k * N_NEGS:(k + 1) * N_NEGS]
                            .unsqueeze(1).to_broadcast([128, HOPS, N_NEGS]),
                        op=Alu.mult)
                    nc.vector.tensor_reduce(out=candf[:, k * HOPS:(k + 1) * HOPS], in_=idsel[:],
                                            axis=mybir.AxisListType.X,
                                            op=Alu.add)

                cand_i = smallp.tile([128, K * HOPS], i32, tag="candi")
                nc.vector.tensor_copy(cand_i[:], candf[:])

                if stage <= 4:
                    continue
                # ---- gather selected rows ----
                selr = selp.tile([128, K * HOPS, ROW], f32, tag="selr")
                for j in range(K * HOPS):
                    nc.gpsimd.indirect_dma_start(
                        out=selr[:, j], out_offset=None,
                        in_=item_tab,
                        in_offset=bass.IndirectOffsetOnAxis(
                            ap=cand_i[:, j:j + 1], axis=0))

                if stage <= 5:
                    continue
                # ---- hop sums ----
                u_sum = smallp.tile([128, DIM], f32, tag="usum")
                p_sum = smallp.tile([128, DIM], f32, tag="psum")
                nc.vector.tensor_reduce(
                    out=u_sum[:],
                    in_=s_t[:].rearrange("p (h d) -> p h d",
                                         h=HOPS).transpose([0, 2, 1]),
                    axis=mybir.AxisListType.X, op=Alu.add)
                nc.vector.tensor_reduce(
                    out=p_sum[:],
                    in_=p_t[:].rearrange("p (h d) -> p h d",
                                         h=HOPS).transpose([0, 2, 1]),
                    axis=mybir.AxisListType.X, op=Alu.add)

                psum_seed = smallp.tile([128, DIM], f32, tag="pseed")
                nc.vector.tensor_scalar_mul(psum_seed[:], p_sum[:], seed_ap)

                # n_sum_k = oms * sum_h selr[k,h-diag] + seed * p_sum
                n_sums = []
                for k in range(K):
                    r_k = smallp.tile([128, DIM], f32, tag=f"rk{k}")
                    nc.vector.tensor_add(
                        r_k[:], selr[:, 4 * k + 0, 0:DIM],
                        selr[:, 4 * k + 1, DIM:2 * DIM])
                    nc.vector.tensor_add(r_k[:], r_k[:],
                                         selr[:, 4 * k + 2, 2 * DIM:3 * DIM])
                    nc.vector.tensor_add(r_k[:], r_k[:],
                                         selr[:, 4 * k + 3, 3 * DIM:4 * DIM])
                    n_k = smallp.tile([128, DIM], f32, tag=f"nk{k}")
                    nc.vector.tensor_scalar_mul(n_k[:], r_k[:], oms_ap)
                    nc.vector.tensor_add(n_k[:], n_k[:], psum_seed[:])
                    n_sums.append(n_k)

                if stage <= 6:
                    continue
                # ---- loss pieces (scale 1/16 folds the /HOPS means) ----
                S = 1.0 / (HOPS * HOPS)
                tmp = smallp.tile([128, DIM], f32, tag="tmp")

                def dotp(out_ap, a, b):
                    nc.vector.tensor_tensor(out=tmp[:], in0=a, in1=b,
                                            op=Alu.mult)
                    nc.vector.tensor_reduce(out=out_ap, in_=tmp[:],
                                            axis=mybir.AxisListType.X,
                                            op=Alu.add)

                pos_s = smallp.tile([128, 1], f32, tag="poss")   # raw (x16)
                dotp(pos_s[:], u_sum[:], p_sum[:])
                neg_s = []
                for k in range(K):
                    ns = smallp.tile([128, 1], f32, tag=f"negs{k}")
                    dotp(ns[:], u_sum[:], n_sums[k][:])
                    neg_s.append(ns)

                sq = smallp.tile([128, 4], f32, tag="sq")        # raw (x16)
                dotp(sq[:, 0:1], u_sum[:], u_sum[:])
                dotp(sq[:, 1:2], p_sum[:], p_sum[:])
                dotp(sq[:, 2:3], n_sums[0][:], n_sums[0][:])
                dotp(sq[:, 3:4], n_sums[1][:], n_sums[1][:])
                sq_tot = smallp.tile([128, 1], f32, tag="sqtot")
                nc.vector.tensor_add(sq_tot[:], sq[:, 0:1], sq[:, 1:2])
                nc.vector.tensor_add(sq_tot[:], sq_tot[:], sq[:, 2:3])
                nc.vector.tensor_add(sq_tot[:], sq_tot[:], sq[:, 3:4])
                nc.vector.tensor_scalar_mul(sq_tot[:], sq_tot[:], S)

                negpos = smallp.tile([128, 1], f32, tag="negpos")
                nc.vector.tensor_scalar_mul(negpos[:], pos_s[:], -S)
                e01 = smallp.tile([128, 2], f32, tag="e01")
                for k in range(K):
                    nc.scalar.activation(out=e01[:, k:k + 1],
                                         in_=neg_s[k][:], func=Act.Exp,
                                         bias=negpos[:], scale=S)
                if stage <= 8:
                    continue
                esum = smallp.tile([128, 1], f32, tag="esum")
                nc.vector.tensor_add(esum[:], e01[:, 0:1], e01[:, 1:2])
                mf = smallp.tile([128, 1], f32, tag="mf")
                nc.scalar.activation(out=mf[:], in_=esum[:], func=Act.Ln,
                                     bias=1.0, scale=1.0)

                nc.vector.tensor_add(part_t[:, 0:1], part_t[:, 0:1], mf[:])
                nc.vector.tensor_add(part_t[:, 1:2], part_t[:, 1:2],
                                     sq_tot[:])

            nc.sync.dma_start(part, part_t[:])
    nc.compile()
    return nc


def _build_runner(nc):
    import jax
    from jax.sharding import Mesh, PartitionSpec
    from jax.experimental.shard_map import shard_map
    from concourse import mybir
    from concourse.bass2jax import (install_neuronx_cc_hook,
                                    partition_id_tensor, _bass_exec_p)

    install_neuronx_cc_hook()
    partition_name = (nc.partition_id_tensor.name
                      if nc.partition_id_tensor else None)
    REPLICATED = {"user_tab", "item_tab"}

    in_names, out_names, out_avals, zero_outs = [], [], [], []
    for alloc in nc.m.functions[0].allocations:
        if not isinstance(alloc, mybir.MemoryLocationSet):
            continue
        name = alloc.memorylocations[0].name
        if alloc.kind == "ExternalInput":
            if name != partition_name:
                in_names.append(name)
        elif alloc.kind == "ExternalOutput":
            out_names.append(name)
            shape = tuple(alloc.tensor_shape)
            dtype = mybir.dt.np(alloc.dtype)
            out_avals.append(jax.core.ShapedArray(shape, dtype))
            zero_outs.append(np.zeros(shape, dtype))
    n_params = len(in_names)
    n_outs = len(out_avals)
    all_in_names = list(in_names) + list(out_names)
    if partition_name is not None:
        all_in_names.append(partition_name)

    def _body(*args):
        operands = list(args)
        if partition_name is not None:
            operands.append(partition_id_tensor())
        outs = _bass_exec_p.bind(
            *operands, out_avals=tuple(out_avals),
            in_names=tuple(all_in_names), out_names=tuple(out_names),
            lowering_input_output_aliases=(), sim_require_finite=True,
            sim_require_nnan=True, nc=nc)
        return tuple(outs)

    devices = jax.devices()[:NCORES]
    mesh = Mesh(np.asarray(devices), ("core",))
    spec_of = [
        PartitionSpec() if name in REPLICATED else PartitionSpec("core")
        for name in in_names
    ]
    in_specs = tuple(spec_of) + (PartitionSpec("core"),) * n_outs
    out_specs = (PartitionSpec("core"),) * n_outs
    sharded = jax.jit(
        shard_map(_body, mesh=mesh, in_specs=in_specs, out_specs=out_specs,
                  check_rep=False),
        keep_unused=True)
    shard_s = jax.sharding.NamedSharding(mesh, PartitionSpec("core"))
    repl_s = jax.sharding.NamedSharding(mesh, PartitionSpec())

    def run(per_core_maps, replicated_map):
        args = []
        for i, name in enumerate(in_names):
            if name in REPLICATED:
                args.append(jax.device_put(replicated_map[name], repl_s))
            else:
                args.append(jax.device_put(
                    np.concatenate([m[name] for m in per_core_maps], axis=0),
                    shard_s))
        for z in zero_outs:
            args.append(jax.device_put(
                np.zeros((NCORES * z.shape[0], *z.shape[1:]), z.dtype),
                shard_s))
        outs = sharded(*args)
        return [
            {name: np.asarray(outs[i]).reshape(NCORES, *out_avals[i].shape)[c]
             for i, name in enumerate(out_names)}
            for c in range(NCORES)
        ]

    return run


def _get_runner():
    import os
    if "run" not in _CACHE:
        nc = _build_bass(int(os.environ.get("KSTAGE", "99")))
        _CACHE["nc"] = nc
        _CACHE["run"] = _build_runner(nc)
    return _CACHE["run"]


def make_in_maps(user_gcn_emb, item_gcn_emb, seed_embed, user, pos_item,
                 neg_item):
    """Host-side sharding/marshalling into per-core input maps."""
    user = np.asarray(user).astype(np.int32)
    pos_item = np.asarray(pos_item).astype(np.int32)
    neg_item = np.asarray(neg_item).astype(np.int32)
    seed = np.asarray(seed_embed, dtype=np.float32).reshape(BATCH)
    per_core = []
    for c in range(NCORES):
        lo = c * B_LOC
        # partition-major: [128 partitions, NCHUNK]
        u = user[lo:lo + B_LOC].reshape(NCHUNK, 128).T.copy()
        p = pos_item[lo:lo + B_LOC].reshape(NCHUNK, 128).T.copy()
        n = (neg_item[lo:lo + B_LOC]
             .reshape(NCHUNK, 128, NCAND).transpose(1, 0, 2).copy())
        s = seed[lo:lo + B_LOC].reshape(NCHUNK, 128).T.copy()
        per_core.append({"uid": u, "pid": p, "nid": n, "seed": s})
    replicated = {
        "user_tab": np.ascontiguousarray(
            np.asarray(user_gcn_emb, dtype=np.float32).reshape(N_USERS, ROW)),
        "item_tab": np.ascontiguousarray(
            np.asarray(item_gcn_emb, dtype=np.float32).reshape(N_ITEMS, ROW)),
    }
    return per_core, replicated


def combine(results):
    mf_sum = 0.0
    sq_sum = 0.0
    for r in results:
        mf_sum += float(r["part"][:, 0].astype(np.float64).sum())
        sq_sum += float(r["part"][:, 1].astype(np.float64).sum())
    mf_loss = np.float32(mf_sum / BATCH)
    emb_loss = np.float32(DECAY * sq_sum / 2.0 / BATCH)
    loss = np.float32(mf_loss + emb_loss)
    return loss, mf_loss, emb_loss


def kernel(user_gcn_emb, item_gcn_emb, seed_embed, user, pos_item, neg_item):
    run = _get_runner()
    per_core, replicated = make_in_maps(user_gcn_emb, item_gcn_emb,
                                        seed_embed, user, pos_item, neg_item)
    results = run(per_core, replicated)
    return combine(results)



# revision 3
# speedup vs baseline: 1.1417x; 1.1417x over previous
"""MixGCF negative-sampling + BPR loss kernel for 8x Trainium2 NeuronCores.

Strategy (data-parallel over batch, per-core row-sharded item subtables):
  - 8 cores x 256 users each (2 chunks of 128 users = partitions).
  - Host dedups each core's 32768 candidate item ids (np.unique -> at most
    32768 unique rows, so remapped ids always fit int16), ships per-core f32
    and bf16 subtables plus remapped indices; user/pos rows are pre-gathered
    densely on host (one row per batch element).
  - Device: per half-chunk (64 candidates x 128 users) one 8192-row
    dma_gather from the bf16 subtable; scores via bf16 multiply (DVE 2x
    mode) + an add-tree over the 64 dims (last levels in f32); per-hop
    argmax via max/iota-onehot; per-hop 64-float slices of the selected
    rows gathered from the f32 subtable; BPR loss in f32 reduced to
    per-partition partial sums. Host combines 8x[128,2] partials.
"""
import sys

sys.path.insert(0, "/opt/trn_rl_repo")
import numpy as np
import ml_dtypes

N_USERS = 200000
N_ITEMS = 200000
HOPS = 4
DIM = 64
BATCH = 2048
N_NEGS = 64
K = 2
DECAY = 1e-4
NCORES = 8
ROW = HOPS * DIM          # 256 f32 per table row
B_LOC = BATCH // NCORES   # 256 users per core
NCHUNK = B_LOC // 128     # 2 chunks of 128 users
NCAND = K * N_NEGS        # 128 candidates per user
NHALF = NCHUNK * 2        # 4 half-chunks of 64 candidates
SUB = NCAND // 2          # 64 candidates per half-chunk
NIDX = SUB * 128          # 8192 gathered rows per dma_gather
USUB = 32768              # per-core item subtable rows (padded)

_CACHE = {}


def _build_bass(stage=99):
    import concourse.bass as bass
    import concourse.tile as tile
    from concourse import bacc, mybir

    f32 = mybir.dt.float32
    bf16 = mybir.dt.bfloat16
    i32 = mybir.dt.int32
    i16 = mybir.dt.int16
    Alu = mybir.AluOpType
    Act = mybir.ActivationFunctionType

    nc = bacc.Bacc("TRN2", target_bir_lowering=False, debug=False,
                   num_devices=NCORES)
    subf = nc.dram_tensor("subf", [USUB, ROW], f32, kind="ExternalInput").ap()
    subb = nc.dram_tensor("subb", [USUB, ROW], bf16,
                          kind="ExternalInput").ap()
    s_rows = nc.dram_tensor("s_rows", [128, NCHUNK, ROW], f32,
                            kind="ExternalInput").ap()
    p_rows = nc.dram_tensor("p_rows", [128, NCHUNK, ROW], f32,
                            kind="ExternalInput").ap()
    cand16 = nc.dram_tensor("cand16", [128, NHALF, NIDX // 16], i16,
                            kind="ExternalInput").ap()
    ridf = nc.dram_tensor("ridf", [128, NCHUNK, NCAND], f32,
                          kind="ExternalInput").ap()
    seed = nc.dram_tensor("seed", [128, NCHUNK], f32,
                          kind="ExternalInput").ap()
    iotas = nc.dram_tensor("iotas", [128, N_NEGS + K * HOPS], f32,
                           kind="ExternalInput").ap()
    part = nc.dram_tensor("part", [128, 2], f32, kind="ExternalOutput").ap()

    subf4 = subf.rearrange("u (h d) -> (u h) d", h=HOPS)

    with tile.TileContext(nc) as tc:
        with tc.tile_pool(name="meta", bufs=1) as meta, \
             tc.tile_pool(name="gat", bufs=3) as gatp, \
             tc.tile_pool(name="tree", bufs=1) as treep, \
             tc.tile_pool(name="sn", bufs=2) as snp, \
             tc.tile_pool(name="sel", bufs=2) as selp, \
             tc.tile_pool(name="small", bufs=2) as smallp:
            # ---- static/meta staging ----
            sp_t = meta.tile([128, NCHUNK, ROW], f32)
            pp_t = meta.tile([128, NCHUNK, ROW], f32)
            cand_t = meta.tile([128, NHALF, NIDX // 16], i16)
            ridf_t = meta.tile([128, NCHUNK, NCAND], f32)
            seed_t = meta.tile([128, NCHUNK], f32)
            iota_t = meta.tile([128, N_NEGS + K * HOPS], f32)
            nc.sync.dma_start(cand_t[:], cand16)
            nc.sync.dma_start(sp_t[:], s_rows)
            nc.sync.dma_start(pp_t[:], p_rows)
            nc.sync.dma_start(ridf_t[:], ridf)
            nc.sync.dma_start(seed_t[:], seed)
            nc.sync.dma_start(iota_t[:], iotas)
            iota_rev = iota_t[:, 0:N_NEGS]
            hpat = iota_t[:, N_NEGS:N_NEGS + K * HOPS]

            oms_t = meta.tile([128, NCHUNK], f32)   # 1 - seed
            nc.vector.tensor_scalar(oms_t[:], seed_t[:], -1.0, 1.0,
                                    Alu.mult, Alu.add)
            s_bf = meta.tile([128, NCHUNK, ROW], bf16)
            nc.vector.tensor_copy(s_bf[:], sp_t[:])

            part_t = meta.tile([128, 2], f32)
            nc.vector.memset(part_t[:], 0.0)

            # ---- candidate gathers: all 4 half-chunks up front ----
            gats = []
            for hc in range(NHALF if stage >= 2 else 0):
                gat = gatp.tile([128, SUB, ROW], bf16, tag="gat")
                nc.gpsimd.dma_gather(
                    out_ap=gat[:], in_ap=subb, idxs_ap=cand_t[:, hc],
                    num_idxs=NIDX, num_idxs_reg=NIDX, elem_size=ROW,
                    single_packet=False)
                gats.append(gat)

            def score_half(ch, hs, sn_t):
                """bf16 mult by user row + add-tree over d -> sn slice."""
                gat = gats[ch * 2 + hs]
                nc.vector.tensor_tensor(
                    out=gat[:], in0=gat[:],
                    in1=s_bf[:, ch].unsqueeze(1).to_broadcast(
                        [128, SUB, ROW]),
                    op=Alu.mult)
                v = gat[:].rearrange("p s (h d) -> p s h d", h=HOPS)
                t32 = treep.tile([128, SUB, HOPS, 32], bf16, tag="t32")
                nc.vector.tensor_tensor(out=t32[:], in0=v[:, :, :, 0:32],
                                        in1=v[:, :, :, 32:64], op=Alu.add)
                t16 = treep.tile([128, SUB, HOPS, 16], bf16, tag="t16")
                nc.vector.tensor_tensor(out=t16[:], in0=t32[:, :, :, 0:16],
                                        in1=t32[:, :, :, 16:32], op=Alu.add)
                t8 = treep.tile([128, SUB, HOPS, 8], bf16, tag="t8")
                nc.vector.tensor_tensor(out=t8[:], in0=t16[:, :, :, 0:8],
                                        in1=t16[:, :, :, 8:16], op=Alu.add)
                t4 = treep.tile([128, SUB, HOPS, 4], f32, tag="t4")
                nc.vector.tensor_tensor(out=t4[:], in0=t8[:, :, :, 0:4],
                                        in1=t8[:, :, :, 4:8], op=Alu.add)
                t2 = treep.tile([128, SUB, HOPS, 2], f32, tag="t2")
                nc.vector.tensor_tensor(out=t2[:], in0=t4[:, :, :, 0:2],
                                        in1=t4[:, :, :, 2:4], op=Alu.add)
                nc.vector.tensor_tensor(
                    out=sn_t[:, hs * SUB:(hs + 1) * SUB, :].unsqueeze(3),
                    in0=t2[:, :, :, 0:1], in1=t2[:, :, :, 1:2], op=Alu.add)

            def argmax_chunk(ch, sn_t):
                """per-(k,hop) argmax -> remapped candidate id (f32)."""
                oms_ap = oms_t[:, ch:ch + 1]
                g_t = snp.tile([128, NCAND, HOPS], f32, tag="g")
                nc.vector.tensor_scalar_mul(g_t[:], sn_t[:], oms_ap)
                candf = smallp.tile([128, K * HOPS], f32, tag="candf")
                for k in range(K):
                    gk = g_t[:, k * N_NEGS:(k + 1) * N_NEGS, :].transpose(
                        [0, 2, 1])                     # [128, H, N]
                    m_k = smallp.tile([128, HOPS], f32, tag="mk")
                    nc.vector.tensor_reduce(out=m_k[:], in_=gk,
                                            axis=mybir.AxisListType.X,
                                            op=Alu.max)
                    eq = smallp.tile([128, HOPS, N_NEGS], f32, tag="eq")
                    nc.vector.tensor_tensor(
                        out=eq[:], in0=gk,
                        in1=m_k[:].unsqueeze(2).to_broadcast(
                            [128, HOPS, N_NEGS]),
                        op=Alu.is_equal)
                    w = smallp.tile([128, HOPS, N_NEGS], f32, tag="w")
                    nc.vector.tensor_tensor(
                        out=w[:], in0=eq[:],
                        in1=iota_rev.unsqueeze(1).to_broadcast(
                            [128, HOPS, N_NEGS]),
                        op=Alu.mult)
                    wmax = smallp.tile([128, HOPS], f32, tag="wmax")
                    nc.vector.tensor_reduce(out=wmax[:], in_=w[:],
                                            axis=mybir.AxisListType.X,
                                            op=Alu.max)
                    onehot = smallp.tile([128, HOPS, N_NEGS], f32, tag="oh")
                    nc.vector.tensor_tensor(
                        out=onehot[:],
                        in0=iota_rev.unsqueeze(1).to_broadcast(
                            [128, HOPS, N_NEGS]),
                        in1=wmax[:].unsqueeze(2).to_broadcast(
                            [128, HOPS, N_NEGS]),
                        op=Alu.is_equal)
                    idsel = smallp.tile([128, HOPS, N_NEGS], f32, tag="ids")
                    nc.vector.tensor_tensor(
                        out=idsel[:], in0=onehot[:],
                        in1=ridf_t[:, ch, k * N_NEGS:(k + 1) * N_NEGS]
                            .unsqueeze(1).to_broadcast([128, HOPS, N_NEGS]),
                        op=Alu.mult)
                    nc.vector.tensor_reduce(
                        out=candf[:, k * HOPS:(k + 1) * HOPS], in_=idsel[:],
                        axis=mybir.AxisListType.X, op=Alu.add)
                # idx into [USUB*HOPS, DIM] view: 4*rid + h
                idx4f = smallp.tile([128, K * HOPS], f32, tag="idx4f")
                nc.vector.tensor_scalar(idx4f[:], candf[:], float(HOPS), 0.0,
                                        Alu.mult, Alu.add)
                nc.vector.tensor_tensor(out=idx4f[:], in0=idx4f[:],
                                        in1=hpat, op=Alu.add)
                cand4_i = smallp.tile([128, K * HOPS], i32, tag="cand4i")
                nc.vector.tensor_copy(cand4_i[:], idx4f[:])
                return cand4_i

            def sel_gather(cand4_i):
                selr = selp.tile([128, K * HOPS, DIM], f32, tag="selr")
                for j in range(K * HOPS):
                    nc.gpsimd.indirect_dma_start(
                        out=selr[:, j], out_offset=None,
                        in_=subf4,
                        in_offset=bass.IndirectOffsetOnAxis(
                            ap=cand4_i[:, j:j + 1], axis=0))
                return selr

            def loss_chunk(ch, selr):
                seed_ap = seed_t[:, ch:ch + 1]
                oms_ap = oms_t[:, ch:ch + 1]
                u_sum = smallp.tile([128, DIM], f32, tag="usum")
                p_sum = smallp.tile([128, DIM], f32, tag="psum")
                nc.vector.tensor_reduce(
                    out=u_sum[:],
                    in_=sp_t[:, ch].rearrange("p (h d) -> p h d",
                                              h=HOPS).transpose([0, 2, 1]),
                    axis=mybir.AxisListType.X, op=Alu.add)
                nc.vector.tensor_reduce(
                    out=p_sum[:],
                    in_=pp_t[:, ch].rearrange("p (h d) -> p h d",
                                              h=HOPS).transpose([0, 2, 1]),
                    axis=mybir.AxisListType.X, op=Alu.add)
                psum_seed = smallp.tile([128, DIM], f32, tag="pseed")
                nc.vector.tensor_scalar_mul(psum_seed[:], p_sum[:], seed_ap)

                n_sums = []
                for k in range(K):
                    r_k = smallp.tile([128, DIM], f32, tag=f"rk{k}")
                    nc.vector.tensor_add(r_k[:], selr[:, 4 * k + 0],
                                         selr[:, 4 * k + 1])
                    nc.vector.tensor_add(r_k[:], r_k[:], selr[:, 4 * k + 2])
                    nc.vector.tensor_add(r_k[:], r_k[:], selr[:, 4 * k + 3])
                    n_k = smallp.tile([128, DIM], f32, tag=f"nk{k}")
                    nc.vector.tensor_scalar_mul(n_k[:], r_k[:], oms_ap)
                    nc.vector.tensor_add(n_k[:], n_k[:], psum_seed[:])
                    n_sums.append(n_k)

                # ---- loss pieces (scale 1/16 folds the /HOPS means) ----
                S = 1.0 / (HOPS * HOPS)
                tmp = smallp.tile([128, DIM], f32, tag="tmp")

                def dotp(out_ap, a, b):
                    nc.vector.tensor_tensor(out=tmp[:], in0=a, in1=b,
                                            op=Alu.mult)
                    nc.vector.tensor_reduce(out=out_ap, in_=tmp[:],
                                            axis=mybir.AxisListType.X,
                                            op=Alu.add)

                pos_s = smallp.tile([128, 1], f32, tag="poss")   # raw (x16)
                dotp(pos_s[:], u_sum[:], p_sum[:])
                neg_s = []
                for k in range(K):
                    ns = smallp.tile([128, 1], f32, tag=f"negs{k}")
                    dotp(ns[:], u_sum[:], n_sums[k][:])
                    neg_s.append(ns)

                sq = smallp.tile([128, 4], f32, tag="sq")        # raw (x16)
                dotp(sq[:, 0:1], u_sum[:], u_sum[:])
                dotp(sq[:, 1:2], p_sum[:], p_sum[:])
                dotp(sq[:, 2:3], n_sums[0][:], n_sums[0][:])
                dotp(sq[:, 3:4], n_sums[1][:], n_sums[1][:])
                sq_tot = smallp.tile([128, 1], f32, tag="sqtot")
                nc.vector.tensor_add(sq_tot[:], sq[:, 0:1], sq[:, 1:2])
                nc.vector.tensor_add(sq_tot[:], sq_tot[:], sq[:, 2:3])
                nc.vector.tensor_add(sq_tot[:], sq_tot[:], sq[:, 3:4])
                nc.vector.tensor_scalar_mul(sq_tot[:], sq_tot[:], S)

                negpos = smallp.tile([128, 1], f32, tag="negpos")
                nc.vector.tensor_scalar_mul(negpos[:], pos_s[:], -S)
                e01 = smallp.tile([128, 2], f32, tag="e01")
                for k in range(K):
                    nc.scalar.activation(out=e01[:, k:k + 1],
                                         in_=neg_s[k][:], func=Act.Exp,
                                         bias=negpos[:], scale=S)
                esum = smallp.tile([128, 1], f32, tag="esum")
                nc.vector.tensor_add(esum[:], e01[:, 0:1], e01[:, 1:2])
                mf = smallp.tile([128, 1], f32, tag="mf")
                nc.scalar.activation(out=mf[:], in_=esum[:], func=Act.Ln,
                                     bias=1.0, scale=1.0)

                nc.vector.tensor_add(part_t[:, 0:1], part_t[:, 0:1], mf[:])
                nc.vector.tensor_add(part_t[:, 1:2], part_t[:, 1:2],
                                     sq_tot[:])

            if stage >= 3:
                sn_ts = [snp.tile([128, NCAND, HOPS], f32, tag="sn",
                                  name=f"sn{i}") for i in range(NCHUNK)]
                # chunk 0 scoring
                score_half(0, 0, sn_ts[0])
                score_half(0, 1, sn_ts[0])
                if stage >= 4:
                    c4_0 = argmax_chunk(0, sn_ts[0])
                    selr0 = sel_gather(c4_0) if stage >= 5 else None
                # chunk 1 scoring (overlaps selr0 gather)
                score_half(1, 0, sn_ts[1])
                score_half(1, 1, sn_ts[1])
                if stage >= 7:
                    loss_chunk(0, selr0)
                if stage >= 4:
                    c4_1 = argmax_chunk(1, sn_ts[1])
                    selr1 = sel_gather(c4_1) if stage >= 5 else None
                if stage >= 7:
                    loss_chunk(1, selr1)

            nc.sync.dma_start(part, part_t[:])
    nc.compile()
    return nc


def _build_runner(nc):
    import jax
    from jax.sharding import Mesh, PartitionSpec
    from jax.experimental.shard_map import shard_map
    from concourse import mybir
    from concourse.bass2jax import (install_neuronx_cc_hook,
                                    partition_id_tensor, _bass_exec_p)

    install_neuronx_cc_hook()
    partition_name = (nc.partition_id_tensor.name
                      if nc.partition_id_tensor else None)
    REPLICATED = set()

    in_names, out_names, out_avals, zero_outs = [], [], [], []
    for alloc in nc.m.functions[0].allocations:
        if not isinstance(alloc, mybir.MemoryLocationSet):
            continue
        name = alloc.memorylocations[0].name
        if alloc.kind == "ExternalInput":
            if name != partition_name:
                in_names.append(name)
        elif alloc.kind == "ExternalOutput":
            out_names.append(name)
            shape = tuple(alloc.tensor_shape)
            dtype = mybir.dt.np(alloc.dtype)
            out_avals.append(jax.core.ShapedArray(shape, dtype))
            zero_outs.append(np.zeros(shape, dtype))
    n_outs = len(out_avals)
    all_in_names = list(in_names) + list(out_names)
    if partition_name is not None:
        all_in_names.append(partition_name)

    def _body(*args):
        operands = list(args)
        if partition_name is not None:
            operands.append(partition_id_tensor())
        outs = _bass_exec_p.bind(
            *operands, out_avals=tuple(out_avals),
            in_names=tuple(all_in_names), out_names=tuple(out_names),
            lowering_input_output_aliases=(), sim_require_finite=True,
            sim_require_nnan=True, nc=nc)
        return tuple(outs)

    devices = jax.devices()[:NCORES]
    mesh = Mesh(np.asarray(devices), ("core",))
    spec_of = [
        PartitionSpec() if name in REPLICATED else PartitionSpec("core")
        for name in in_names
    ]
    in_specs = tuple(spec_of) + (PartitionSpec("core"),) * n_outs
    out_specs = (PartitionSpec("core"),) * n_outs
    sharded = jax.jit(
        shard_map(_body, mesh=mesh, in_specs=in_specs, out_specs=out_specs,
                  check_rep=False),
        keep_unused=True)
    shard_s = jax.sharding.NamedSharding(mesh, PartitionSpec("core"))
    repl_s = jax.sharding.NamedSharding(mesh, PartitionSpec())

    def run(per_core_maps, replicated_map):
        args = []
        for name in in_names:
            if name in REPLICATED:
                args.append(jax.device_put(replicated_map[name], repl_s))
            else:
                args.append(jax.device_put(
                    np.concatenate([m[name] for m in per_core_maps], axis=0),
                    shard_s))
        for z in zero_outs:
            args.append(jax.device_put(
                np.zeros((NCORES * z.shape[0], *z.shape[1:]), z.dtype),
                shard_s))
        outs = sharded(*args)
        return [
            {name: np.asarray(outs[i]).reshape(NCORES, *out_avals[i].shape)[c]
             for i, name in enumerate(out_names)}
            for c in range(NCORES)
        ]

    return run


def _get_runner():
    import os
    if "run" not in _CACHE:
        nc = _build_bass(int(os.environ.get("KSTAGE", "99")))
        _CACHE["nc"] = nc
        _CACHE["run"] = _build_runner(nc)
    return _CACHE["run"]


def _wrap_idx(arr):
    """dma_gather index layout: position i -> channel i%16, col i//16;
    the [16, n/16] block is replicated to all 128 partitions."""
    n = arr.shape[0]
    w = arr.reshape(n // 16, 16).T.astype(np.int16)
    return np.tile(w, (8, 1))


def make_in_maps(user_gcn_emb, item_gcn_emb, seed_embed, user, pos_item,
                 neg_item):
    """Host-side sharding/marshalling into per-core input maps."""
    user = np.asarray(user).astype(np.int64)
    pos_item = np.asarray(pos_item).astype(np.int64)
    neg_item = np.asarray(neg_item).astype(np.int64)
    seed = np.asarray(seed_embed, dtype=np.float32).reshape(BATCH)
    utab = np.ascontiguousarray(
        np.asarray(user_gcn_emb, dtype=np.float32).reshape(N_USERS, ROW))
    itab = np.ascontiguousarray(
        np.asarray(item_gcn_emb, dtype=np.float32).reshape(N_ITEMS, ROW))

    iota_block = np.tile(
        np.concatenate([
            (N_NEGS - np.arange(N_NEGS)).astype(np.float32),
            np.tile(np.arange(HOPS, dtype=np.float32), K)]),
        (128, 1))

    per_core = []
    for c in range(NCORES):
        lo = c * B_LOC
        u = user[lo:lo + B_LOC]
        p = pos_item[lo:lo + B_LOC]
        nid = neg_item[lo:lo + B_LOC]                   # [256, 128]
        s_rows = utab[u].reshape(NCHUNK, 128, ROW).transpose(1, 0, 2).copy()
        p_rows = itab[p].reshape(NCHUNK, 128, ROW).transpose(1, 0, 2).copy()
        uniq, inv = np.unique(nid, return_inverse=True)
        rid = inv.reshape(B_LOC, NCAND)                 # values < len(uniq)
        subf = np.zeros((USUB, ROW), dtype=np.float32)
        subf[:len(uniq)] = itab[uniq]
        subb = subf.astype(ml_dtypes.bfloat16)
        cand = np.empty((128, NHALF, NIDX // 16), dtype=np.int16)
        for ch in range(NCHUNK):
            for hs in range(2):
                arr = rid[ch * 128:(ch + 1) * 128,
                          hs * SUB:(hs + 1) * SUB].T.ravel()
                cand[:, ch * 2 + hs, :] = _wrap_idx(arr)
        ridf = (rid.reshape(NCHUNK, 128, NCAND).transpose(1, 0, 2)
                .astype(np.float32))
        s = seed[lo:lo + B_LOC].reshape(NCHUNK, 128).T.copy()
        per_core.append({
            "subf": subf, "subb": subb, "s_rows": s_rows, "p_rows": p_rows,
            "cand16": cand, "ridf": ridf, "seed": s, "iotas": iota_block,
        })
    return per_core, {}


def combine(results):
    mf_sum = 0.0
    sq_sum = 0.0
    for r in results:
        mf_sum += float(r["part"][:, 0].astype(np.float64).sum())
        sq_sum += float(r["part"][:, 1].astype(np.float64).sum())
    mf_loss = np.float32(mf_sum / BATCH)
    emb_loss = np.float32(DECAY * sq_sum / 2.0 / BATCH)
    loss = np.float32(mf_loss + emb_loss)
    return loss, mf_loss, emb_loss


def kernel(user_gcn_emb, item_gcn_emb, seed_embed, user, pos_item, neg_item):
    run = _get_runner()
    per_core, replicated = make_in_maps(user_gcn_emb, item_gcn_emb,
                                        seed_embed, user, pos_item, neg_item)
    results = run(per_core, replicated)
    return combine(results)


# revision 5
# speedup vs baseline: 1.1579x; 1.0142x over previous
"""MixGCF negative-sampling + BPR loss kernel for 8x Trainium2 NeuronCores.

Strategy (data-parallel over batch, per-core row-sharded item subtables):
  - 8 cores x 256 users each (2 chunks of 128 users = partitions).
  - Host dedups each core's 32768 candidate item ids (np.unique -> at most
    32768 unique rows, so remapped ids always fit int16), ships per-core f32
    and bf16 subtables plus remapped indices; user/pos rows are pre-gathered
    densely on host (one row per batch element).
  - Device: per half-chunk (64 candidates x 128 users) one 8192-row
    dma_gather from the bf16 subtable; scores via bf16 multiply (DVE 2x
    mode) + an add-tree over the 64 dims (last levels in f32); per-hop
    argmax via max/iota-onehot; per-hop 64-float slices of the selected
    rows gathered from the f32 subtable; BPR loss in f32 reduced to
    per-partition partial sums. Host combines 8x[128,2] partials.
"""
import sys

sys.path.insert(0, "/opt/trn_rl_repo")
import numpy as np
import ml_dtypes

N_USERS = 200000
N_ITEMS = 200000
HOPS = 4
DIM = 64
BATCH = 2048
N_NEGS = 64
K = 2
DECAY = 1e-4
NCORES = 8
ROW = HOPS * DIM          # 256 f32 per table row
B_LOC = BATCH // NCORES   # 256 users per core
NCHUNK = B_LOC // 128     # 2 chunks of 128 users
NCAND = K * N_NEGS        # 128 candidates per user
NHALF = NCHUNK * 2        # 4 half-chunks of 64 candidates
SUB = NCAND // 2          # 64 candidates per half-chunk
NIDX = SUB * 128          # 8192 gathered rows per dma_gather
USUB = 32768              # per-core item subtable rows (padded)

_CACHE = {}


def _build_bass(stage=99):
    import concourse.bass as bass
    import concourse.tile as tile
    from concourse import bacc, mybir

    f32 = mybir.dt.float32
    bf16 = mybir.dt.bfloat16
    i32 = mybir.dt.int32
    i16 = mybir.dt.int16
    Alu = mybir.AluOpType
    Act = mybir.ActivationFunctionType

    nc = bacc.Bacc("TRN2", target_bir_lowering=False, debug=False,
                   num_devices=NCORES, dynamic_dma_scratch_size=32768)
    subf = nc.dram_tensor("subf", [USUB, ROW], f32, kind="ExternalInput").ap()
    subb = nc.dram_tensor("subb", [USUB, ROW], bf16,
                          kind="ExternalInput").ap()
    s_rows = nc.dram_tensor("s_rows", [128, NCHUNK, ROW], f32,
                            kind="ExternalInput").ap()
    p_rows = nc.dram_tensor("p_rows", [128, NCHUNK, ROW], f32,
                            kind="ExternalInput").ap()
    cand16 = nc.dram_tensor("cand16", [128, NHALF, NIDX // 16], i16,
                            kind="ExternalInput").ap()
    ridf = nc.dram_tensor("ridf", [128, NCHUNK, NCAND], f32,
                          kind="ExternalInput").ap()
    seed = nc.dram_tensor("seed", [128, NCHUNK], f32,
                          kind="ExternalInput").ap()
    iotas = nc.dram_tensor("iotas", [128, N_NEGS + K * HOPS], f32,
                           kind="ExternalInput").ap()
    part = nc.dram_tensor("part", [128, 2], f32, kind="ExternalOutput").ap()

    subf4 = subf.rearrange("u (h d) -> (u h) d", h=HOPS)

    with tile.TileContext(nc) as tc:
        with tc.tile_pool(name="meta", bufs=1) as meta, \
             tc.tile_pool(name="gat", bufs=2) as gatp, \
             tc.tile_pool(name="tree", bufs=1) as treep, \
             tc.tile_pool(name="sn", bufs=2) as snp, \
             tc.tile_pool(name="sel", bufs=2) as selp, \
             tc.tile_pool(name="small", bufs=2) as smallp:
            # ---- static/meta staging ----
            sp_t = meta.tile([128, NCHUNK, ROW], f32)
            pp_t = meta.tile([128, NCHUNK, ROW], f32)
            cand_t = meta.tile([128, NHALF, NIDX // 16], i16)
            ridf_t = meta.tile([128, NCHUNK, NCAND], f32)
            seed_t = meta.tile([128, NCHUNK], f32)
            iota_t = meta.tile([128, N_NEGS + K * HOPS], f32)
            nc.sync.dma_start(cand_t[:], cand16)
            nc.sync.dma_start(sp_t[:], s_rows)
            nc.sync.dma_start(pp_t[:], p_rows)
            nc.sync.dma_start(ridf_t[:], ridf)
            nc.sync.dma_start(seed_t[:], seed)
            nc.sync.dma_start(iota_t[:], iotas)
            iota_rev = iota_t[:, 0:N_NEGS]
            hpat = iota_t[:, N_NEGS:N_NEGS + K * HOPS]

            oms_t = meta.tile([128, NCHUNK], f32)   # 1 - seed
            nc.vector.tensor_scalar(oms_t[:], seed_t[:], -1.0, 1.0,
                                    Alu.mult, Alu.add)
            s_bf = meta.tile([128, NCHUNK, ROW], bf16)
            nc.vector.tensor_copy(s_bf[:], sp_t[:])

            part_t = meta.tile([128, 2], f32)
            nc.vector.memset(part_t[:], 0.0)

            # ---- candidate gathers: all 4 half-chunks up front ----
            gats = []
            for hc in range(NHALF if stage >= 2 else 0):
                gat = gatp.tile([128, SUB, ROW], bf16, tag="gat")
                nc.gpsimd.dma_gather(
                    out_ap=gat[:], in_ap=subb, idxs_ap=cand_t[:, hc],
                    num_idxs=NIDX, num_idxs_reg=NIDX, elem_size=ROW,
                    single_packet=False)
                gats.append(gat)

            def score_half(ch, hs, sn_t):
                """bf16 mult by user row + add-tree over d -> sn slice."""
                gat = gats[ch * 2 + hs]
                nc.vector.tensor_tensor(
                    out=gat[:], in0=gat[:],
                    in1=s_bf[:, ch].unsqueeze(1).to_broadcast(
                        [128, SUB, ROW]),
                    op=Alu.mult)
                v = gat[:].rearrange("p s (h d) -> p s h d", h=HOPS)
                t32 = treep.tile([128, SUB, HOPS, 32], bf16, tag="t32")
                nc.vector.tensor_tensor(out=t32[:], in0=v[:, :, :, 0:32],
                                        in1=v[:, :, :, 32:64], op=Alu.add)
                t16 = treep.tile([128, SUB, HOPS, 16], bf16, tag="t16")
                nc.vector.tensor_tensor(out=t16[:], in0=t32[:, :, :, 0:16],
                                        in1=t32[:, :, :, 16:32], op=Alu.add)
                t8 = treep.tile([128, SUB, HOPS, 8], bf16, tag="t8")
                nc.vector.tensor_tensor(out=t8[:], in0=t16[:, :, :, 0:8],
                                        in1=t16[:, :, :, 8:16], op=Alu.add)
                t4 = treep.tile([128, SUB, HOPS, 4], f32, tag="t4")
                nc.vector.tensor_tensor(out=t4[:], in0=t8[:, :, :, 0:4],
                                        in1=t8[:, :, :, 4:8], op=Alu.add)
                t2 = treep.tile([128, SUB, HOPS, 2], f32, tag="t2")
                nc.vector.tensor_tensor(out=t2[:], in0=t4[:, :, :, 0:2],
                                        in1=t4[:, :, :, 2:4], op=Alu.add)
                nc.vector.tensor_tensor(
                    out=sn_t[:, hs * SUB:(hs + 1) * SUB, :].unsqueeze(3),
                    in0=t2[:, :, :, 0:1], in1=t2[:, :, :, 1:2], op=Alu.add)

            def argmax_chunk(ch, sn_t):
                """per-(k,hop) argmax -> remapped candidate id (f32)."""
                oms_ap = oms_t[:, ch:ch + 1]
                g_t = snp.tile([128, NCAND, HOPS], f32, tag="g")
                nc.vector.tensor_scalar_mul(g_t[:], sn_t[:], oms_ap)
                candf = smallp.tile([128, K * HOPS], f32, tag="candf")
                for k in range(K):
                    gk = g_t[:, k * N_NEGS:(k + 1) * N_NEGS, :].transpose(
                        [0, 2, 1])                     # [128, H, N]
                    m_k = smallp.tile([128, HOPS], f32, tag="mk")
                    nc.vector.tensor_reduce(out=m_k[:], in_=gk,
                                            axis=mybir.AxisListType.X,
                                            op=Alu.max)
                    eq = smallp.tile([128, HOPS, N_NEGS], f32, tag="eq")
                    nc.vector.tensor_tensor(
                        out=eq[:], in0=gk,
                        in1=m_k[:].unsqueeze(2).to_broadcast(
                            [128, HOPS, N_NEGS]),
                        op=Alu.is_equal)
                    w = smallp.tile([128, HOPS, N_NEGS], f32, tag="w")
                    nc.vector.tensor_tensor(
                        out=w[:], in0=eq[:],
                        in1=iota_rev.unsqueeze(1).to_broadcast(
                            [128, HOPS, N_NEGS]),
                        op=Alu.mult)
                    wmax = smallp.tile([128, HOPS], f32, tag="wmax")
                    nc.vector.tensor_reduce(out=wmax[:], in_=w[:],
                                            axis=mybir.AxisListType.X,
                                            op=Alu.max)
                    onehot = smallp.tile([128, HOPS, N_NEGS], f32, tag="oh")
                    nc.vector.tensor_tensor(
                        out=onehot[:],
                        in0=iota_rev.unsqueeze(1).to_broadcast(
                            [128, HOPS, N_NEGS]),
                        in1=wmax[:].unsqueeze(2).to_broadcast(
                            [128, HOPS, N_NEGS]),
                        op=Alu.is_equal)
                    idsel = smallp.tile([128, HOPS, N_NEGS], f32, tag="ids")
                    nc.vector.tensor_tensor(
                        out=idsel[:], in0=onehot[:],
                        in1=ridf_t[:, ch, k * N_NEGS:(k + 1) * N_NEGS]
                            .unsqueeze(1).to_broadcast([128, HOPS, N_NEGS]),
                        op=Alu.mult)
                    nc.vector.tensor_reduce(
                        out=candf[:, k * HOPS:(k + 1) * HOPS], in_=idsel[:],
                        axis=mybir.AxisListType.X, op=Alu.add)
                # idx into [USUB*HOPS, DIM] view: 4*rid + h
                idx4f = smallp.tile([128, K * HOPS], f32, tag="idx4f")
                nc.vector.tensor_scalar(idx4f[:], candf[:], float(HOPS), 0.0,
                                        Alu.mult, Alu.add)
                nc.vector.tensor_tensor(out=idx4f[:], in0=idx4f[:],
                                        in1=hpat, op=Alu.add)
                cand4_i = smallp.tile([128, K * HOPS], i32, tag="cand4i")
                nc.vector.tensor_copy(cand4_i[:], idx4f[:])
                return cand4_i

            def sel_gather(cand4_i):
                selr = selp.tile([128, K * HOPS, DIM], f32, tag="selr")
                for j in range(K * HOPS):
                    nc.gpsimd.indirect_dma_start(
                        out=selr[:, j], out_offset=None,
                        in_=subf4,
                        in_offset=bass.IndirectOffsetOnAxis(
                            ap=cand4_i[:, j:j + 1], axis=0))
                return selr

            def loss_chunk(ch, selr):
                seed_ap = seed_t[:, ch:ch + 1]
                oms_ap = oms_t[:, ch:ch + 1]
                u_sum = smallp.tile([128, DIM], f32, tag="usum")
                p_sum = smallp.tile([128, DIM], f32, tag="psum")
                nc.vector.tensor_reduce(
                    out=u_sum[:],
                    in_=sp_t[:, ch].rearrange("p (h d) -> p h d",
                                              h=HOPS).transpose([0, 2, 1]),
                    axis=mybir.AxisListType.X, op=Alu.add)
                nc.vector.tensor_reduce(
                    out=p_sum[:],
                    in_=pp_t[:, ch].rearrange("p (h d) -> p h d",
                                              h=HOPS).transpose([0, 2, 1]),
                    axis=mybir.AxisListType.X, op=Alu.add)
                psum_seed = smallp.tile([128, DIM], f32, tag="pseed")
                nc.vector.tensor_scalar_mul(psum_seed[:], p_sum[:], seed_ap)

                n_sums = []
                for k in range(K):
                    r_k = smallp.tile([128, DIM], f32, tag=f"rk{k}")
                    nc.vector.tensor_add(r_k[:], selr[:, 4 * k + 0],
                                         selr[:, 4 * k + 1])
                    nc.vector.tensor_add(r_k[:], r_k[:], selr[:, 4 * k + 2])
                    nc.vector.tensor_add(r_k[:], r_k[:], selr[:, 4 * k + 3])
                    n_k = smallp.tile([128, DIM], f32, tag=f"nk{k}")
                    nc.vector.tensor_scalar_mul(n_k[:], r_k[:], oms_ap)
                    nc.vector.tensor_add(n_k[:], n_k[:], psum_seed[:])
                    n_sums.append(n_k)

                # ---- loss pieces (scale 1/16 folds the /HOPS means) ----
                S = 1.0 / (HOPS * HOPS)
                tmp = smallp.tile([128, DIM], f32, tag="tmp")

                def dotp(out_ap, a, b):
                    nc.vector.tensor_tensor(out=tmp[:], in0=a, in1=b,
                                            op=Alu.mult)
                    nc.vector.tensor_reduce(out=out_ap, in_=tmp[:],
                                            axis=mybir.AxisListType.X,
                                            op=Alu.add)

                pos_s = smallp.tile([128, 1], f32, tag="poss")   # raw (x16)
                dotp(pos_s[:], u_sum[:], p_sum[:])
                neg_s = []
                for k in range(K):
                    ns = smallp.tile([128, 1], f32, tag=f"negs{k}")
                    dotp(ns[:], u_sum[:], n_sums[k][:])
                    neg_s.append(ns)

                sq = smallp.tile([128, 4], f32, tag="sq")        # raw (x16)
                dotp(sq[:, 0:1], u_sum[:], u_sum[:])
                dotp(sq[:, 1:2], p_sum[:], p_sum[:])
                dotp(sq[:, 2:3], n_sums[0][:], n_sums[0][:])
                dotp(sq[:, 3:4], n_sums[1][:], n_sums[1][:])
                sq_tot = smallp.tile([128, 1], f32, tag="sqtot")
                nc.vector.tensor_add(sq_tot[:], sq[:, 0:1], sq[:, 1:2])
                nc.vector.tensor_add(sq_tot[:], sq_tot[:], sq[:, 2:3])
                nc.vector.tensor_add(sq_tot[:], sq_tot[:], sq[:, 3:4])
                nc.vector.tensor_scalar_mul(sq_tot[:], sq_tot[:], S)

                negpos = smallp.tile([128, 1], f32, tag="negpos")
                nc.vector.tensor_scalar_mul(negpos[:], pos_s[:], -S)
                e01 = smallp.tile([128, 2], f32, tag="e01")
                for k in range(K):
                    nc.scalar.activation(out=e01[:, k:k + 1],
                                         in_=neg_s[k][:], func=Act.Exp,
                                         bias=negpos[:], scale=S)
                esum = smallp.tile([128, 1], f32, tag="esum")
                nc.vector.tensor_add(esum[:], e01[:, 0:1], e01[:, 1:2])
                mf = smallp.tile([128, 1], f32, tag="mf")
                nc.scalar.activation(out=mf[:], in_=esum[:], func=Act.Ln,
                                     bias=1.0, scale=1.0)

                nc.vector.tensor_add(part_t[:, 0:1], part_t[:, 0:1], mf[:])
                nc.vector.tensor_add(part_t[:, 1:2], part_t[:, 1:2],
                                     sq_tot[:])

            if stage >= 3:
                sn_ts = [snp.tile([128, NCAND, HOPS], f32, tag="sn",
                                  name=f"sn{i}") for i in range(NCHUNK)]
                # chunk 0 scoring
                score_half(0, 0, sn_ts[0])
                score_half(0, 1, sn_ts[0])
                if stage >= 4:
                    c4_0 = argmax_chunk(0, sn_ts[0])
                    selr0 = sel_gather(c4_0) if stage >= 5 else None
                # chunk 1 scoring (overlaps selr0 gather)
                score_half(1, 0, sn_ts[1])
                score_half(1, 1, sn_ts[1])
                if stage >= 7:
                    loss_chunk(0, selr0)
                if stage >= 4:
                    c4_1 = argmax_chunk(1, sn_ts[1])
                    selr1 = sel_gather(c4_1) if stage >= 5 else None
                if stage >= 7:
                    loss_chunk(1, selr1)

            nc.sync.dma_start(part, part_t[:])
    nc.compile()
    return nc


def _build_runner(nc):
    import jax
    from jax.sharding import Mesh, PartitionSpec
    from jax.experimental.shard_map import shard_map
    from concourse import mybir
    from concourse.bass2jax import (install_neuronx_cc_hook,
                                    partition_id_tensor, _bass_exec_p)

    install_neuronx_cc_hook()
    partition_name = (nc.partition_id_tensor.name
                      if nc.partition_id_tensor else None)
    REPLICATED = set()

    in_names, out_names, out_avals, zero_outs = [], [], [], []
    for alloc in nc.m.functions[0].allocations:
        if not isinstance(alloc, mybir.MemoryLocationSet):
            continue
        name = alloc.memorylocations[0].name
        if alloc.kind == "ExternalInput":
            if name != partition_name:
                in_names.append(name)
        elif alloc.kind == "ExternalOutput":
            out_names.append(name)
            shape = tuple(alloc.tensor_shape)
            dtype = mybir.dt.np(alloc.dtype)
            out_avals.append(jax.core.ShapedArray(shape, dtype))
            zero_outs.append(np.zeros(shape, dtype))
    n_outs = len(out_avals)
    all_in_names = list(in_names) + list(out_names)
    if partition_name is not None:
        all_in_names.append(partition_name)

    def _body(*args):
        operands = list(args)
        if partition_name is not None:
            operands.append(partition_id_tensor())
        outs = _bass_exec_p.bind(
            *operands, out_avals=tuple(out_avals),
            in_names=tuple(all_in_names), out_names=tuple(out_names),
            lowering_input_output_aliases=(), sim_require_finite=True,
            sim_require_nnan=True, nc=nc)
        return tuple(outs)

    devices = jax.devices()[:NCORES]
    mesh = Mesh(np.asarray(devices), ("core",))
    spec_of = [
        PartitionSpec() if name in REPLICATED else PartitionSpec("core")
        for name in in_names
    ]
    in_specs = tuple(spec_of) + (PartitionSpec("core"),) * n_outs
    out_specs = (PartitionSpec("core"),) * n_outs
    sharded = jax.jit(
        shard_map(_body, mesh=mesh, in_specs=in_specs, out_specs=out_specs,
                  check_rep=False),
        keep_unused=True)
    shard_s = jax.sharding.NamedSharding(mesh, PartitionSpec("core"))
    repl_s = jax.sharding.NamedSharding(mesh, PartitionSpec())

    def run(per_core_maps, replicated_map):
        args = []
        for name in in_names:
            if name in REPLICATED:
                args.append(jax.device_put(replicated_map[name], repl_s))
            else:
                args.append(jax.device_put(
                    np.concatenate([m[name] for m in per_core_maps], axis=0),
                    shard_s))
        for z in zero_outs:
            args.append(jax.device_put(
                np.zeros((NCORES * z.shape[0], *z.shape[1:]), z.dtype),
                shard_s))
        outs = sharded(*args)
        return [
            {name: np.asarray(outs[i]).reshape(NCORES, *out_avals[i].shape)[c]
             for i, name in enumerate(out_names)}
            for c in range(NCORES)
        ]

    return run


def _get_runner():
    import os
    if "run" not in _CACHE:
        nc = _build_bass(int(os.environ.get("KSTAGE", "99")))
        _CACHE["nc"] = nc
        _CACHE["run"] = _build_runner(nc)
    return _CACHE["run"]


def _wrap_idx(arr):
    """dma_gather index layout: position i -> channel i%16, col i//16;
    the [16, n/16] block is replicated to all 128 partitions."""
    n = arr.shape[0]
    w = arr.reshape(n // 16, 16).T.astype(np.int16)
    return np.tile(w, (8, 1))


def make_in_maps(user_gcn_emb, item_gcn_emb, seed_embed, user, pos_item,
                 neg_item):
    """Host-side sharding/marshalling into per-core input maps."""
    user = np.asarray(user).astype(np.int64)
    pos_item = np.asarray(pos_item).astype(np.int64)
    neg_item = np.asarray(neg_item).astype(np.int64)
    seed = np.asarray(seed_embed, dtype=np.float32).reshape(BATCH)
    utab = np.ascontiguousarray(
        np.asarray(user_gcn_emb, dtype=np.float32).reshape(N_USERS, ROW))
    itab = np.ascontiguousarray(
        np.asarray(item_gcn_emb, dtype=np.float32).reshape(N_ITEMS, ROW))

    iota_block = np.tile(
        np.concatenate([
            (N_NEGS - np.arange(N_NEGS)).astype(np.float32),
            np.tile(np.arange(HOPS, dtype=np.float32), K)]),
        (128, 1))

    per_core = []
    for c in range(NCORES):
        lo = c * B_LOC
        u = user[lo:lo + B_LOC]
        p = pos_item[lo:lo + B_LOC]
        nid = neg_item[lo:lo + B_LOC]                   # [256, 128]
        s_rows = utab[u].reshape(NCHUNK, 128, ROW).transpose(1, 0, 2).copy()
        p_rows = itab[p].reshape(NCHUNK, 128, ROW).transpose(1, 0, 2).copy()
        uniq, inv = np.unique(nid, return_inverse=True)
        rid = inv.reshape(B_LOC, NCAND)                 # values < len(uniq)
        subf = np.zeros((USUB, ROW), dtype=np.float32)
        subf[:len(uniq)] = itab[uniq]
        subb = subf.astype(ml_dtypes.bfloat16)
        cand = np.empty((128, NHALF, NIDX // 16), dtype=np.int16)
        for ch in range(NCHUNK):
            for hs in range(2):
                arr = rid[ch * 128:(ch + 1) * 128,
                          hs * SUB:(hs + 1) * SUB].T.ravel()
                cand[:, ch * 2 + hs, :] = _wrap_idx(arr)
        ridf = (rid.reshape(NCHUNK, 128, NCAND).transpose(1, 0, 2)
                .astype(np.float32))
        s = seed[lo:lo + B_LOC].reshape(NCHUNK, 128).T.copy()
        per_core.append({
            "subf": subf, "subb": subb, "s_rows": s_rows, "p_rows": p_rows,
            "cand16": cand, "ridf": ridf, "seed": s, "iotas": iota_block,
        })
    return per_core, {}


def combine(results):
    mf_sum = 0.0
    sq_sum = 0.0
    for r in results:
        mf_sum += float(r["part"][:, 0].astype(np.float64).sum())
        sq_sum += float(r["part"][:, 1].astype(np.float64).sum())
    mf_loss = np.float32(mf_sum / BATCH)
    emb_loss = np.float32(DECAY * sq_sum / 2.0 / BATCH)
    loss = np.float32(mf_loss + emb_loss)
    return loss, mf_loss, emb_loss


def kernel(user_gcn_emb, item_gcn_emb, seed_embed, user, pos_item, neg_item):
    run = _get_runner()
    per_core, replicated = make_in_maps(user_gcn_emb, item_gcn_emb,
                                        seed_embed, user, pos_item, neg_item)
    results = run(per_core, replicated)
    return combine(results)


# revision 13
# speedup vs baseline: 1.4279x; 1.2332x over previous
"""MixGCF negative-sampling + BPR loss kernel for 8x Trainium2 NeuronCores.

Strategy (data-parallel over batch, per-core row-sharded item subtables):
  - 8 cores x 256 users each (2 chunks of 128 users = partitions).
  - Host dedups each core's 32768 candidate item ids (np.unique -> at most
    32768 unique rows, so remapped ids always fit int16), ships per-core f32
    and bf16 subtables plus remapped indices; user/pos rows are pre-gathered
    densely on host (one row per batch element).
  - Device: per half-chunk (64 candidates x 128 users) one 8192-row
    dma_gather from the bf16 subtable; scores via bf16 multiply (DVE 2x
    mode) + an add-tree over the 64 dims (last levels in f32); per-hop
    argmax via max/iota-onehot; per-hop 64-float slices of the selected
    rows gathered from the f32 subtable; BPR loss in f32 reduced to
    per-partition partial sums. Host combines 8x[128,2] partials.
"""
import sys

sys.path.insert(0, "/opt/trn_rl_repo")
import numpy as np
import ml_dtypes

N_USERS = 200000
N_ITEMS = 200000
HOPS = 4
DIM = 64
BATCH = 2048
N_NEGS = 64
K = 2
DECAY = 1e-4
NCORES = 8
ROW = HOPS * DIM          # 256 f32 per table row
B_LOC = BATCH // NCORES   # 256 users per core
NCHUNK = B_LOC // 128     # 2 chunks of 128 users
NCAND = K * N_NEGS        # 128 candidates per user
NQ = 4                    # quarter-chunks per chunk (32 candidates each)
SUB = NCAND // NQ         # 32 candidates per quarter
NIDX = SUB * 128          # 4096 gathered rows per dma_gather
USUB = 32768              # per-core item subtable rows (padded)

_CACHE = {}


def _build_bass(stage=99):
    import concourse.bass as bass
    import concourse.tile as tile
    from concourse import bacc, mybir

    f32 = mybir.dt.float32
    bf16 = mybir.dt.bfloat16
    i32 = mybir.dt.int32
    i16 = mybir.dt.int16
    Alu = mybir.AluOpType
    Act = mybir.ActivationFunctionType

    nc = bacc.Bacc("TRN2", target_bir_lowering=False, debug=False,
                   num_devices=NCORES, dynamic_dma_scratch_size=32768)
    subf = nc.dram_tensor("subf", [USUB, ROW], f32, kind="ExternalInput").ap()
    subb = nc.dram_tensor("subb", [USUB, ROW], bf16,
                          kind="ExternalInput").ap()
    s_rows = nc.dram_tensor("s_rows", [128, NCHUNK, ROW], f32,
                            kind="ExternalInput").ap()
    p_rows = nc.dram_tensor("p_rows", [128, NCHUNK, ROW], f32,
                            kind="ExternalInput").ap()
    cand16 = nc.dram_tensor("cand16", [128, NCHUNK * NQ, NIDX // 16], i16,
                            kind="ExternalInput").ap()
    ridf = nc.dram_tensor("ridf", [128, NCHUNK, NCAND], f32,
                          kind="ExternalInput").ap()
    seed = nc.dram_tensor("seed", [128, NCHUNK], f32,
                          kind="ExternalInput").ap()
    iotas = nc.dram_tensor("iotas", [128, N_NEGS + HOPS], f32,
                           kind="ExternalInput").ap()
    part = nc.dram_tensor("part", [128, 2], f32, kind="ExternalOutput").ap()

    subf4 = subf.rearrange("u (h d) -> (u h) d", h=HOPS)

    with tile.TileContext(nc) as tc:
        with tc.tile_pool(name="meta", bufs=1) as meta, \
             tc.tile_pool(name="gat", bufs=3) as gatp, \
             tc.tile_pool(name="tree", bufs=1) as treep, \
             tc.tile_pool(name="sn", bufs=4) as snp, \
             tc.tile_pool(name="sel", bufs=4) as selp, \
             tc.tile_pool(name="small", bufs=2) as smallp:
            # ---- static/meta staging ----
            sp_t = meta.tile([128, NCHUNK, ROW], f32)
            pp_t = meta.tile([128, NCHUNK, ROW], f32)
            cand_t = meta.tile([128, NCHUNK * NQ, NIDX // 16], i16)
            ridf_t = meta.tile([128, NCHUNK, NCAND], f32)
            seed_t = meta.tile([128, NCHUNK], f32)
            iota_t = meta.tile([128, N_NEGS + HOPS], f32)
            nc.sync.dma_start(cand_t[:], cand16)
            nc.sync.dma_start(sp_t[:], s_rows)
            nc.sync.dma_start(pp_t[:], p_rows)
            nc.sync.dma_start(ridf_t[:], ridf)
            nc.sync.dma_start(seed_t[:], seed)
            nc.sync.dma_start(iota_t[:], iotas)
            iota_rev = iota_t[:, 0:N_NEGS]
            hpat = iota_t[:, N_NEGS:N_NEGS + HOPS]

            oms_t = meta.tile([128, NCHUNK], f32)   # 1 - seed
            nc.vector.tensor_scalar(oms_t[:], seed_t[:], -1.0, 1.0,
                                    Alu.mult, Alu.add)
            s_bf = meta.tile([128, NCHUNK, ROW], bf16)
            nc.vector.tensor_copy(s_bf[:], sp_t[:])

            part_t = meta.tile([128, 2], f32)
            nc.vector.memset(part_t[:], 0.0)

            # ---- hoisted loss prep (dense inputs only) ----
            u_sum = meta.tile([128, NCHUNK, DIM], f32)
            p_sum = meta.tile([128, NCHUNK, DIM], f32)
            psum_seed = meta.tile([128, NCHUNK, DIM], f32)
            for ch in range(NCHUNK):
                nc.vector.tensor_reduce(
                    out=u_sum[:, ch],
                    in_=sp_t[:, ch].rearrange("p (h d) -> p h d",
                                              h=HOPS).transpose([0, 2, 1]),
                    axis=mybir.AxisListType.X, op=Alu.add)
                nc.vector.tensor_reduce(
                    out=p_sum[:, ch],
                    in_=pp_t[:, ch].rearrange("p (h d) -> p h d",
                                              h=HOPS).transpose([0, 2, 1]),
                    axis=mybir.AxisListType.X, op=Alu.add)
                nc.vector.tensor_scalar_mul(psum_seed[:, ch], p_sum[:, ch],
                                            seed_t[:, ch:ch + 1])

            def emit_gather(ch, q):
                gat = gatp.tile([128, SUB, ROW], bf16, tag="gat",
                                name=f"gat{ch}{q}")
                nc.gpsimd.dma_gather(
                    out_ap=gat[:], in_ap=subb,
                    idxs_ap=cand_t[:, ch * NQ + q],
                    num_idxs=NIDX, num_idxs_reg=NIDX, elem_size=ROW,
                    single_packet=False)
                return gat

            def score_quarter(ch, q, gat, sn_k):
                """bf16 mult by user row + add-tree over d -> sn_k slice."""
                qq = q % 2
                nc.vector.tensor_tensor(
                    out=gat[:], in0=gat[:],
                    in1=s_bf[:, ch].unsqueeze(1).to_broadcast(
                        [128, SUB, ROW]),
                    op=Alu.mult)
                v = gat[:].rearrange("p s (h d) -> p s h d", h=HOPS)
                t32 = treep.tile([128, SUB, HOPS, 32], bf16, tag="t32")
                nc.vector.tensor_tensor(out=t32[:], in0=v[:, :, :, 0:32],
                                        in1=v[:, :, :, 32:64], op=Alu.add)
                t16 = treep.tile([128, SUB, HOPS, 16], bf16, tag="t16")
                nc.vector.tensor_tensor(out=t16[:], in0=t32[:, :, :, 0:16],
                                        in1=t32[:, :, :, 16:32], op=Alu.add)
                t8 = treep.tile([128, SUB, HOPS, 8], bf16, tag="t8")
                nc.vector.tensor_tensor(out=t8[:], in0=t16[:, :, :, 0:8],
                                        in1=t16[:, :, :, 8:16], op=Alu.add)
                t4 = treep.tile([128, SUB, HOPS, 4], f32, tag="t4")
                nc.vector.tensor_tensor(out=t4[:], in0=t8[:, :, :, 0:4],
                                        in1=t8[:, :, :, 4:8], op=Alu.add)
                t2 = treep.tile([128, SUB, HOPS, 2], f32, tag="t2")
                nc.vector.tensor_tensor(out=t2[:], in0=t4[:, :, :, 0:2],
                                        in1=t4[:, :, :, 2:4], op=Alu.add)
                nc.vector.tensor_tensor(
                    out=sn_k[:, qq * SUB:(qq + 1) * SUB, :].unsqueeze(3),
                    in0=t2[:, :, :, 0:1], in1=t2[:, :, :, 1:2], op=Alu.add)

            def argmax_half(ch, k, sn_k):
                """per-hop argmax over this k-half -> 4*rid+h (i32)."""
                oms_ap = oms_t[:, ch:ch + 1]
                g_t = snp.tile([128, N_NEGS, HOPS], f32, tag="g")
                nc.vector.tensor_scalar_mul(g_t[:], sn_k[:], oms_ap)
                gk = g_t[:].transpose([0, 2, 1])       # [128, H, N]
                m_k = smallp.tile([128, HOPS], f32, tag="mk")
                nc.vector.tensor_reduce(out=m_k[:], in_=gk,
                                        axis=mybir.AxisListType.X,
                                        op=Alu.max)
                eq = smallp.tile([128, HOPS, N_NEGS], f32, tag="eq")
                nc.vector.tensor_tensor(
                    out=eq[:], in0=gk,
                    in1=m_k[:].unsqueeze(2).to_broadcast(
                        [128, HOPS, N_NEGS]),
                    op=Alu.is_equal)
                w = smallp.tile([128, HOPS, N_NEGS], f32, tag="w")
                nc.vector.tensor_tensor(
                    out=w[:], in0=eq[:],
                    in1=iota_rev.unsqueeze(1).to_broadcast(
                        [128, HOPS, N_NEGS]),
                    op=Alu.mult)
                wmax = smallp.tile([128, HOPS], f32, tag="wmax")
                nc.vector.tensor_reduce(out=wmax[:], in_=w[:],
                                        axis=mybir.AxisListType.X,
                                        op=Alu.max)
                onehot = smallp.tile([128, HOPS, N_NEGS], f32, tag="oh")
                nc.vector.tensor_tensor(
                    out=onehot[:],
                    in0=iota_rev.unsqueeze(1).to_broadcast(
                        [128, HOPS, N_NEGS]),
                    in1=wmax[:].unsqueeze(2).to_broadcast(
                        [128, HOPS, N_NEGS]),
                    op=Alu.is_equal)
                idsel = smallp.tile([128, HOPS, N_NEGS], f32, tag="ids")
                nc.vector.tensor_tensor(
                    out=idsel[:], in0=onehot[:],
                    in1=ridf_t[:, ch, k * N_NEGS:(k + 1) * N_NEGS]
                        .unsqueeze(1).to_broadcast([128, HOPS, N_NEGS]),
                    op=Alu.mult)
                candf = smallp.tile([128, HOPS], f32, tag="candf")
                nc.vector.tensor_reduce(out=candf[:], in_=idsel[:],
                                        axis=mybir.AxisListType.X,
                                        op=Alu.add)
                # idx into [USUB*HOPS, DIM] view: 4*rid + h
                idx4f = smallp.tile([128, HOPS], f32, tag="idx4f")
                nc.vector.tensor_scalar(idx4f[:], candf[:], float(HOPS), 0.0,
                                        Alu.mult, Alu.add)
                nc.vector.tensor_tensor(out=idx4f[:], in0=idx4f[:],
                                        in1=hpat, op=Alu.add)
                cand4_i = smallp.tile([128, HOPS], i32, tag="cand4i")
                nc.vector.tensor_copy(cand4_i[:], idx4f[:])
                return cand4_i

            def sel_gather(ch, k, cand4_i):
                selr = selp.tile([128, HOPS, DIM], f32, tag="selr",
                                 name=f"selr{ch}{k}")
                for j in range(HOPS):
                    nc.gpsimd.indirect_dma_start(
                        out=selr[:, j], out_offset=None,
                        in_=subf4,
                        in_offset=bass.IndirectOffsetOnAxis(
                            ap=cand4_i[:, j:j + 1], axis=0))
                return selr

            def loss_chunk(ch, selrs):
                seed_ap = seed_t[:, ch:ch + 1]
                oms_ap = oms_t[:, ch:ch + 1]
                n_sums = []
                for k in range(K):
                    selr = selrs[k]
                    r_k = smallp.tile([128, DIM], f32, tag=f"rk{k}")
                    nc.vector.tensor_add(r_k[:], selr[:, 0], selr[:, 1])
                    nc.vector.tensor_add(r_k[:], r_k[:], selr[:, 2])
                    nc.vector.tensor_add(r_k[:], r_k[:], selr[:, 3])
                    n_k = smallp.tile([128, DIM], f32, tag=f"nk{k}")
                    nc.vector.tensor_scalar_mul(n_k[:], r_k[:], oms_ap)
                    nc.vector.tensor_add(n_k[:], n_k[:], psum_seed[:, ch])
                    n_sums.append(n_k)

                # ---- loss pieces (scale 1/16 folds the /HOPS means) ----
                S = 1.0 / (HOPS * HOPS)
                tmp = smallp.tile([128, DIM], f32, tag="tmp")

                def dotp(out_ap, a, b):
                    nc.vector.tensor_tensor(out=tmp[:], in0=a, in1=b,
                                            op=Alu.mult)
                    nc.vector.tensor_reduce(out=out_ap, in_=tmp[:],
                                            axis=mybir.AxisListType.X,
                                            op=Alu.add)

                pos_s = smallp.tile([128, 1], f32, tag="poss")   # raw (x16)
                dotp(pos_s[:], u_sum[:, ch], p_sum[:, ch])
                neg_s = []
                for k in range(K):
                    ns = smallp.tile([128, 1], f32, tag=f"negs{k}")
                    dotp(ns[:], u_sum[:, ch], n_sums[k][:])
                    neg_s.append(ns)

                sq = smallp.tile([128, 4], f32, tag="sq")        # raw (x16)
                dotp(sq[:, 0:1], u_sum[:, ch], u_sum[:, ch])
                dotp(sq[:, 1:2], p_sum[:, ch], p_sum[:, ch])
                dotp(sq[:, 2:3], n_sums[0][:], n_sums[0][:])
                dotp(sq[:, 3:4], n_sums[1][:], n_sums[1][:])
                sq_tot = smallp.tile([128, 1], f32, tag="sqtot")
                nc.vector.tensor_add(sq_tot[:], sq[:, 0:1], sq[:, 1:2])
                nc.vector.tensor_add(sq_tot[:], sq_tot[:], sq[:, 2:3])
                nc.vector.tensor_add(sq_tot[:], sq_tot[:], sq[:, 3:4])
                nc.vector.tensor_scalar_mul(sq_tot[:], sq_tot[:], S)

                negpos = smallp.tile([128, 1], f32, tag="negpos")
                nc.vector.tensor_scalar_mul(negpos[:], pos_s[:], -S)
                e01 = smallp.tile([128, 2], f32, tag="e01")
                for k in range(K):
                    nc.scalar.activation(out=e01[:, k:k + 1],
                                         in_=neg_s[k][:], func=Act.Exp,
                                         bias=negpos[:], scale=S)
                esum = smallp.tile([128, 1], f32, tag="esum")
                nc.vector.tensor_add(esum[:], e01[:, 0:1], e01[:, 1:2])
                mf = smallp.tile([128, 1], f32, tag="mf")
                nc.scalar.activation(out=mf[:], in_=esum[:], func=Act.Ln,
                                     bias=1.0, scale=1.0)

                nc.vector.tensor_add(part_t[:, 0:1], part_t[:, 0:1], mf[:])
                nc.vector.tensor_add(part_t[:, 1:2], part_t[:, 1:2],
                                     sq_tot[:])

            if stage >= 2:
                # Pool stream: g00..g03, g10, sel(0,0), g11, sel(0,1),
                # g12, g13, sel(1,0), sel(1,1) — sel gathers slot between
                # later desc-gens so only sel(1,1) lands in the tail.
                sn_ts = [snp.tile([128, N_NEGS, HOPS], f32, tag="sn",
                                  name=f"sn{i}") for i in range(NCHUNK * K)]
                g00 = emit_gather(0, 0)
                g01 = emit_gather(0, 1)
                g02 = emit_gather(0, 2)
                g03 = emit_gather(0, 3)
                if stage >= 3:
                    score_quarter(0, 0, g00, sn_ts[0])
                    score_quarter(0, 1, g01, sn_ts[0])
                g10 = emit_gather(1, 0)
                if stage >= 4:
                    c4 = argmax_half(0, 0, sn_ts[0])
                    selr00 = sel_gather(0, 0, c4) if stage >= 5 else None
                if stage >= 3:
                    score_quarter(0, 2, g02, sn_ts[1])
                    score_quarter(0, 3, g03, sn_ts[1])
                g11 = emit_gather(1, 1)
                if stage >= 4:
                    c4 = argmax_half(0, 1, sn_ts[1])
                    selr01 = sel_gather(0, 1, c4) if stage >= 5 else None
                g12 = emit_gather(1, 2)
                g13 = emit_gather(1, 3)
                if stage >= 3:
                    score_quarter(1, 0, g10, sn_ts[2])
                    score_quarter(1, 1, g11, sn_ts[2])
                if stage >= 7:
                    loss_chunk(0, [selr00, selr01])
                if stage >= 4:
                    c4 = argmax_half(1, 0, sn_ts[2])
                    selr10 = sel_gather(1, 0, c4) if stage >= 5 else None
                if stage >= 3:
                    score_quarter(1, 2, g12, sn_ts[3])
                    score_quarter(1, 3, g13, sn_ts[3])
                if stage >= 4:
                    c4 = argmax_half(1, 1, sn_ts[3])
                    selr11 = sel_gather(1, 1, c4) if stage >= 5 else None
                if stage >= 7:
                    loss_chunk(1, [selr10, selr11])

            nc.sync.dma_start(part, part_t[:])
    nc.compile()
    return nc


def _build_runner(nc):
    import jax
    from jax.sharding import Mesh, PartitionSpec
    from jax.experimental.shard_map import shard_map
    from concourse import mybir
    from concourse.bass2jax import (install_neuronx_cc_hook,
                                    partition_id_tensor, _bass_exec_p)

    install_neuronx_cc_hook()
    partition_name = (nc.partition_id_tensor.name
                      if nc.partition_id_tensor else None)
    REPLICATED = set()

    in_names, out_names, out_avals, zero_outs = [], [], [], []
    for alloc in nc.m.functions[0].allocations:
        if not isinstance(alloc, mybir.MemoryLocationSet):
            continue
        name = alloc.memorylocations[0].name
        if alloc.kind == "ExternalInput":
            if name != partition_name:
                in_names.append(name)
        elif alloc.kind == "ExternalOutput":
            out_names.append(name)
            shape = tuple(alloc.tensor_shape)
            dtype = mybir.dt.np(alloc.dtype)
            out_avals.append(jax.core.ShapedArray(shape, dtype))
            zero_outs.append(np.zeros(shape, dtype))
    n_outs = len(out_avals)
    all_in_names = list(in_names) + list(out_names)
    if partition_name is not None:
        all_in_names.append(partition_name)

    def _body(*args):
        operands = list(args)
        if partition_name is not None:
            operands.append(partition_id_tensor())
        outs = _bass_exec_p.bind(
            *operands, out_avals=tuple(out_avals),
            in_names=tuple(all_in_names), out_names=tuple(out_names),
            lowering_input_output_aliases=(), sim_require_finite=True,
            sim_require_nnan=True, nc=nc)
        return tuple(outs)

    devices = jax.devices()[:NCORES]
    mesh = Mesh(np.asarray(devices), ("core",))
    spec_of = [
        PartitionSpec() if name in REPLICATED else PartitionSpec("core")
        for name in in_names
    ]
    in_specs = tuple(spec_of) + (PartitionSpec("core"),) * n_outs
    out_specs = (PartitionSpec("core"),) * n_outs
    sharded = jax.jit(
        shard_map(_body, mesh=mesh, in_specs=in_specs, out_specs=out_specs,
                  check_rep=False),
        keep_unused=True)
    shard_s = jax.sharding.NamedSharding(mesh, PartitionSpec("core"))
    repl_s = jax.sharding.NamedSharding(mesh, PartitionSpec())

    def run(per_core_maps, replicated_map):
        args = []
        for name in in_names:
            if name in REPLICATED:
                args.append(jax.device_put(replicated_map[name], repl_s))
            else:
                args.append(jax.device_put(
                    np.concatenate([m[name] for m in per_core_maps], axis=0),
                    shard_s))
        for z in zero_outs:
            args.append(jax.device_put(
                np.zeros((NCORES * z.shape[0], *z.shape[1:]), z.dtype),
                shard_s))
        outs = sharded(*args)
        return [
            {name: np.asarray(outs[i]).reshape(NCORES, *out_avals[i].shape)[c]
             for i, name in enumerate(out_names)}
            for c in range(NCORES)
        ]

    return run


def _get_runner():
    import os
    if "run" not in _CACHE:
        nc = _build_bass(int(os.environ.get("KSTAGE", "99")))
        _CACHE["nc"] = nc
        _CACHE["run"] = _build_runner(nc)
    return _CACHE["run"]


def _wrap_idx(arr):
    """dma_gather index layout: position i -> channel i%16, col i//16;
    the [16, n/16] block is replicated to all 128 partitions."""
    n = arr.shape[0]
    w = arr.reshape(n // 16, 16).T.astype(np.int16)
    return np.tile(w, (8, 1))


def make_in_maps(user_gcn_emb, item_gcn_emb, seed_embed, user, pos_item,
                 neg_item):
    """Host-side sharding/marshalling into per-core input maps."""
    user = np.asarray(user).astype(np.int64)
    pos_item = np.asarray(pos_item).astype(np.int64)
    neg_item = np.asarray(neg_item).astype(np.int64)
    seed = np.asarray(seed_embed, dtype=np.float32).reshape(BATCH)
    utab = np.ascontiguousarray(
        np.asarray(user_gcn_emb, dtype=np.float32).reshape(N_USERS, ROW))
    itab = np.ascontiguousarray(
        np.asarray(item_gcn_emb, dtype=np.float32).reshape(N_ITEMS, ROW))

    iota_block = np.tile(
        np.concatenate([
            (N_NEGS - np.arange(N_NEGS)).astype(np.float32),
            np.arange(HOPS, dtype=np.float32)]),
        (128, 1))

    per_core = []
    for c in range(NCORES):
        lo = c * B_LOC
        u = user[lo:lo + B_LOC]
        p = pos_item[lo:lo + B_LOC]
        nid = neg_item[lo:lo + B_LOC]                   # [256, 128]
        s_rows = utab[u].reshape(NCHUNK, 128, ROW).transpose(1, 0, 2).copy()
        p_rows = itab[p].reshape(NCHUNK, 128, ROW).transpose(1, 0, 2).copy()
        uniq, inv = np.unique(nid, return_inverse=True)
        rid = inv.reshape(B_LOC, NCAND)                 # values < len(uniq)
        subf = np.zeros((USUB, ROW), dtype=np.float32)
        subf[:len(uniq)] = itab[uniq]
        subb = subf.astype(ml_dtypes.bfloat16)
        cand = np.empty((128, NCHUNK * NQ, NIDX // 16), dtype=np.int16)
        for ch in range(NCHUNK):
            for q in range(NQ):
                arr = rid[ch * 128:(ch + 1) * 128,
                          q * SUB:(q + 1) * SUB].T.ravel()
                cand[:, ch * NQ + q, :] = _wrap_idx(arr)
        ridf = (rid.reshape(NCHUNK, 128, NCAND).transpose(1, 0, 2)
                .astype(np.float32))
        s = seed[lo:lo + B_LOC].reshape(NCHUNK, 128).T.copy()
        per_core.append({
            "subf": subf, "subb": subb, "s_rows": s_rows, "p_rows": p_rows,
            "cand16": cand, "ridf": ridf, "seed": s, "iotas": iota_block,
        })
    return per_core, {}


def combine(results):
    mf_sum = 0.0
    sq_sum = 0.0
    for r in results:
        mf_sum += float(r["part"][:, 0].astype(np.float64).sum())
        sq_sum += float(r["part"][:, 1].astype(np.float64).sum())
    mf_loss = np.float32(mf_sum / BATCH)
    emb_loss = np.float32(DECAY * sq_sum / 2.0 / BATCH)
    loss = np.float32(mf_loss + emb_loss)
    return loss, mf_loss, emb_loss


def kernel(user_gcn_emb, item_gcn_emb, seed_embed, user, pos_item, neg_item):
    run = _get_runner()
    per_core, replicated = make_in_maps(user_gcn_emb, item_gcn_emb,
                                        seed_embed, user, pos_item, neg_item)
    results = run(per_core, replicated)
    return combine(results)


# revision 15
# speedup vs baseline: 1.5137x; 1.0601x over previous
"""MixGCF negative-sampling + BPR loss kernel for 8x Trainium2 NeuronCores.

Strategy (data-parallel over batch, per-core row-sharded item subtables):
  - 8 cores x 256 users each (2 chunks of 128 users = partitions).
  - Host dedups each core's 32768 candidate item ids (np.unique -> at most
    32768 unique rows, so remapped ids always fit int16), ships per-core f32
    and bf16 subtables plus remapped indices; user/pos rows are pre-gathered
    densely on host (one row per batch element).
  - Device: per half-chunk (64 candidates x 128 users) one 8192-row
    dma_gather from the bf16 subtable; scores via bf16 multiply (DVE 2x
    mode) + an add-tree over the 64 dims (last levels in f32); per-hop
    argmax via max/iota-onehot; per-hop 64-float slices of the selected
    rows gathered from the f32 subtable; BPR loss in f32 reduced to
    per-partition partial sums. Host combines 8x[128,2] partials.
"""
import sys

sys.path.insert(0, "/opt/trn_rl_repo")
import numpy as np
import ml_dtypes

N_USERS = 200000
N_ITEMS = 200000
HOPS = 4
DIM = 64
BATCH = 2048
N_NEGS = 64
K = 2
DECAY = 1e-4
NCORES = 8
ROW = HOPS * DIM          # 256 f32 per table row
B_LOC = BATCH // NCORES   # 256 users per core
NCHUNK = B_LOC // 128     # 2 chunks of 128 users
NCAND = K * N_NEGS        # 128 candidates per user
NQ = 4                    # quarter-chunks per chunk (32 candidates each)
SUB = NCAND // NQ         # 32 candidates per quarter
NIDX = SUB * 128          # 4096 gathered rows per dma_gather
USUB = 32768              # per-core item subtable rows (padded)

_CACHE = {}


def _build_bass(stage=99):
    import concourse.bass as bass
    import concourse.tile as tile
    from concourse import bacc, mybir

    f32 = mybir.dt.float32
    bf16 = mybir.dt.bfloat16
    i32 = mybir.dt.int32
    i16 = mybir.dt.int16
    Alu = mybir.AluOpType
    Act = mybir.ActivationFunctionType

    nc = bacc.Bacc("TRN2", target_bir_lowering=False, debug=False,
                   num_devices=NCORES, dynamic_dma_scratch_size=32768)
    subf = nc.dram_tensor("subf", [USUB, ROW], f32, kind="ExternalInput").ap()
    subb = nc.dram_tensor("subb", [USUB, ROW], bf16,
                          kind="ExternalInput").ap()
    s_rows = nc.dram_tensor("s_rows", [128, NCHUNK, ROW], f32,
                            kind="ExternalInput").ap()
    p_rows = nc.dram_tensor("p_rows", [128, NCHUNK, ROW], f32,
                            kind="ExternalInput").ap()
    cand16 = nc.dram_tensor("cand16", [128, NCHUNK * NQ, NIDX // 16], i16,
                            kind="ExternalInput").ap()
    ridf = nc.dram_tensor("ridf", [128, NCHUNK, NCAND], f32,
                          kind="ExternalInput").ap()
    seed = nc.dram_tensor("seed", [128, NCHUNK], f32,
                          kind="ExternalInput").ap()
    iotas = nc.dram_tensor("iotas", [128, N_NEGS + HOPS], f32,
                           kind="ExternalInput").ap()
    part = nc.dram_tensor("part", [128, 2], f32, kind="ExternalOutput").ap()

    subf4 = subf.rearrange("u (h d) -> (u h) d", h=HOPS)

    with tile.TileContext(nc) as tc:
        with tc.tile_pool(name="meta", bufs=1) as meta, \
             tc.tile_pool(name="gat", bufs=4) as gatp, \
             tc.tile_pool(name="tree", bufs=1) as treep, \
             tc.tile_pool(name="sn", bufs=4) as snp, \
             tc.tile_pool(name="sel", bufs=4) as selp, \
             tc.tile_pool(name="small", bufs=2) as smallp:
            # ---- static/meta staging ----
            sp_t = meta.tile([128, NCHUNK, ROW], f32)
            pp_t = meta.tile([128, NCHUNK, ROW], f32)
            cand_t = meta.tile([128, NCHUNK * NQ, NIDX // 16], i16)
            ridf_t = meta.tile([128, NCHUNK, NCAND], f32)
            seed_t = meta.tile([128, NCHUNK], f32)
            iota_t = meta.tile([128, N_NEGS + HOPS], f32)
            nc.sync.dma_start(cand_t[:], cand16)
            nc.sync.dma_start(sp_t[:], s_rows)
            nc.sync.dma_start(pp_t[:], p_rows)
            nc.sync.dma_start(ridf_t[:], ridf)
            nc.sync.dma_start(seed_t[:], seed)
            nc.sync.dma_start(iota_t[:], iotas)
            iota_rev = iota_t[:, 0:N_NEGS]
            hpat = iota_t[:, N_NEGS:N_NEGS + HOPS]

            oms_t = meta.tile([128, NCHUNK], f32)   # 1 - seed
            nc.vector.tensor_scalar(oms_t[:], seed_t[:], -1.0, 1.0,
                                    Alu.mult, Alu.add)
            s_bf = meta.tile([128, NCHUNK, ROW], bf16)
            nc.vector.tensor_copy(s_bf[:], sp_t[:])

            part_t = meta.tile([128, 2], f32)
            nc.vector.memset(part_t[:], 0.0)

            # ---- hoisted loss prep (dense inputs only) ----
            u_sum = meta.tile([128, NCHUNK, DIM], f32)
            p_sum = meta.tile([128, NCHUNK, DIM], f32)
            psum_seed = meta.tile([128, NCHUNK, DIM], f32)
            for ch in range(NCHUNK):
                nc.vector.tensor_reduce(
                    out=u_sum[:, ch],
                    in_=sp_t[:, ch].rearrange("p (h d) -> p h d",
                                              h=HOPS).transpose([0, 2, 1]),
                    axis=mybir.AxisListType.X, op=Alu.add)
                nc.vector.tensor_reduce(
                    out=p_sum[:, ch],
                    in_=pp_t[:, ch].rearrange("p (h d) -> p h d",
                                              h=HOPS).transpose([0, 2, 1]),
                    axis=mybir.AxisListType.X, op=Alu.add)
                nc.vector.tensor_scalar_mul(psum_seed[:, ch], p_sum[:, ch],
                                            seed_t[:, ch:ch + 1])

            def emit_gather(ch, q):
                gat = gatp.tile([128, SUB, ROW], bf16, tag="gat",
                                name=f"gat{ch}{q}")
                nc.gpsimd.dma_gather(
                    out_ap=gat[:], in_ap=subb,
                    idxs_ap=cand_t[:, ch * NQ + q],
                    num_idxs=NIDX, num_idxs_reg=NIDX, elem_size=ROW,
                    single_packet=False)
                return gat

            def score_quarter(ch, q, gat, sn_k):
                """bf16 mult by user row + add-tree over d -> sn_k slice."""
                qq = q % 2
                nc.vector.tensor_tensor(
                    out=gat[:], in0=gat[:],
                    in1=s_bf[:, ch].unsqueeze(1).to_broadcast(
                        [128, SUB, ROW]),
                    op=Alu.mult)
                v = gat[:].rearrange("p s (h d) -> p s h d", h=HOPS)
                t32 = treep.tile([128, SUB, HOPS, 32], bf16, tag="t32")
                nc.vector.tensor_tensor(out=t32[:], in0=v[:, :, :, 0:32],
                                        in1=v[:, :, :, 32:64], op=Alu.add)
                t16 = treep.tile([128, SUB, HOPS, 16], bf16, tag="t16")
                nc.vector.tensor_tensor(out=t16[:], in0=t32[:, :, :, 0:16],
                                        in1=t32[:, :, :, 16:32], op=Alu.add)
                t8 = treep.tile([128, SUB, HOPS, 8], bf16, tag="t8")
                nc.vector.tensor_tensor(out=t8[:], in0=t16[:, :, :, 0:8],
                                        in1=t16[:, :, :, 8:16], op=Alu.add)
                t4 = treep.tile([128, SUB, HOPS, 4], f32, tag="t4")
                nc.vector.tensor_tensor(out=t4[:], in0=t8[:, :, :, 0:4],
                                        in1=t8[:, :, :, 4:8], op=Alu.add)
                t2 = treep.tile([128, SUB, HOPS, 2], f32, tag="t2")
                nc.vector.tensor_tensor(out=t2[:], in0=t4[:, :, :, 0:2],
                                        in1=t4[:, :, :, 2:4], op=Alu.add)
                nc.vector.tensor_tensor(
                    out=sn_k[:, qq * SUB:(qq + 1) * SUB, :].unsqueeze(3),
                    in0=t2[:, :, :, 0:1], in1=t2[:, :, :, 1:2], op=Alu.add)

            def argmax_half(ch, k, sn_k):
                """per-hop argmax over this k-half -> 4*rid+h (i32)."""
                oms_ap = oms_t[:, ch:ch + 1]
                g_t = snp.tile([128, N_NEGS, HOPS], f32, tag="g")
                nc.vector.tensor_scalar_mul(g_t[:], sn_k[:], oms_ap)
                gk = g_t[:].transpose([0, 2, 1])       # [128, H, N]
                m_k = smallp.tile([128, HOPS], f32, tag="mk")
                nc.vector.tensor_reduce(out=m_k[:], in_=gk,
                                        axis=mybir.AxisListType.X,
                                        op=Alu.max)
                eq = smallp.tile([128, HOPS, N_NEGS], f32, tag="eq")
                nc.vector.tensor_tensor(
                    out=eq[:], in0=gk,
                    in1=m_k[:].unsqueeze(2).to_broadcast(
                        [128, HOPS, N_NEGS]),
                    op=Alu.is_equal)
                w = smallp.tile([128, HOPS, N_NEGS], f32, tag="w")
                nc.vector.tensor_tensor(
                    out=w[:], in0=eq[:],
                    in1=iota_rev.unsqueeze(1).to_broadcast(
                        [128, HOPS, N_NEGS]),
                    op=Alu.mult)
                wmax = smallp.tile([128, HOPS], f32, tag="wmax")
                nc.vector.tensor_reduce(out=wmax[:], in_=w[:],
                                        axis=mybir.AxisListType.X,
                                        op=Alu.max)
                onehot = smallp.tile([128, HOPS, N_NEGS], f32, tag="oh")
                nc.vector.tensor_tensor(
                    out=onehot[:],
                    in0=iota_rev.unsqueeze(1).to_broadcast(
                        [128, HOPS, N_NEGS]),
                    in1=wmax[:].unsqueeze(2).to_broadcast(
                        [128, HOPS, N_NEGS]),
                    op=Alu.is_equal)
                idsel = smallp.tile([128, HOPS, N_NEGS], f32, tag="ids")
                nc.vector.tensor_tensor(
                    out=idsel[:], in0=onehot[:],
                    in1=ridf_t[:, ch, k * N_NEGS:(k + 1) * N_NEGS]
                        .unsqueeze(1).to_broadcast([128, HOPS, N_NEGS]),
                    op=Alu.mult)
                candf = smallp.tile([128, HOPS], f32, tag="candf")
                nc.vector.tensor_reduce(out=candf[:], in_=idsel[:],
                                        axis=mybir.AxisListType.X,
                                        op=Alu.add)
                # idx into [USUB*HOPS, DIM] view: 4*rid + h
                idx4f = smallp.tile([128, HOPS], f32, tag="idx4f")
                nc.vector.tensor_scalar(idx4f[:], candf[:], float(HOPS), 0.0,
                                        Alu.mult, Alu.add)
                nc.vector.tensor_tensor(out=idx4f[:], in0=idx4f[:],
                                        in1=hpat, op=Alu.add)
                cand4_i = smallp.tile([128, HOPS], i32, tag="cand4i")
                nc.vector.tensor_copy(cand4_i[:], idx4f[:])
                return cand4_i

            def sel_gather(ch, k, cand4_i):
                selr = selp.tile([128, HOPS, DIM], f32, tag="selr",
                                 name=f"selr{ch}{k}")
                for j in range(HOPS):
                    nc.gpsimd.indirect_dma_start(
                        out=selr[:, j], out_offset=None,
                        in_=subf4,
                        in_offset=bass.IndirectOffsetOnAxis(
                            ap=cand4_i[:, j:j + 1], axis=0))
                return selr

            def loss_chunk(ch, selrs):
                seed_ap = seed_t[:, ch:ch + 1]
                oms_ap = oms_t[:, ch:ch + 1]
                n_sums = []
                for k in range(K):
                    selr = selrs[k]
                    r_k = smallp.tile([128, DIM], f32, tag=f"rk{k}")
                    nc.vector.tensor_add(r_k[:], selr[:, 0], selr[:, 1])
                    nc.vector.tensor_add(r_k[:], r_k[:], selr[:, 2])
                    nc.vector.tensor_add(r_k[:], r_k[:], selr[:, 3])
                    n_k = smallp.tile([128, DIM], f32, tag=f"nk{k}")
                    nc.vector.tensor_scalar_mul(n_k[:], r_k[:], oms_ap)
                    nc.vector.tensor_add(n_k[:], n_k[:], psum_seed[:, ch])
                    n_sums.append(n_k)

                # ---- loss pieces (scale 1/16 folds the /HOPS means) ----
                S = 1.0 / (HOPS * HOPS)
                tmp = smallp.tile([128, DIM], f32, tag="tmp")

                def dotp(out_ap, a, b):
                    nc.vector.tensor_tensor(out=tmp[:], in0=a, in1=b,
                                            op=Alu.mult)
                    nc.vector.tensor_reduce(out=out_ap, in_=tmp[:],
                                            axis=mybir.AxisListType.X,
                                            op=Alu.add)

                pos_s = smallp.tile([128, 1], f32, tag="poss")   # raw (x16)
                dotp(pos_s[:], u_sum[:, ch], p_sum[:, ch])
                neg_s = []
                for k in range(K):
                    ns = smallp.tile([128, 1], f32, tag=f"negs{k}")
                    dotp(ns[:], u_sum[:, ch], n_sums[k][:])
                    neg_s.append(ns)

                sq = smallp.tile([128, 4], f32, tag="sq")        # raw (x16)
                dotp(sq[:, 0:1], u_sum[:, ch], u_sum[:, ch])
                dotp(sq[:, 1:2], p_sum[:, ch], p_sum[:, ch])
                dotp(sq[:, 2:3], n_sums[0][:], n_sums[0][:])
                dotp(sq[:, 3:4], n_sums[1][:], n_sums[1][:])
                sq_tot = smallp.tile([128, 1], f32, tag="sqtot")
                nc.vector.tensor_add(sq_tot[:], sq[:, 0:1], sq[:, 1:2])
                nc.vector.tensor_add(sq_tot[:], sq_tot[:], sq[:, 2:3])
                nc.vector.tensor_add(sq_tot[:], sq_tot[:], sq[:, 3:4])
                nc.vector.tensor_scalar_mul(sq_tot[:], sq_tot[:], S)

                negpos = smallp.tile([128, 1], f32, tag="negpos")
                nc.vector.tensor_scalar_mul(negpos[:], pos_s[:], -S)
                e01 = smallp.tile([128, 2], f32, tag="e01")
                for k in range(K):
                    nc.scalar.activation(out=e01[:, k:k + 1],
                                         in_=neg_s[k][:], func=Act.Exp,
                                         bias=negpos[:], scale=S)
                esum = smallp.tile([128, 1], f32, tag="esum")
                nc.vector.tensor_add(esum[:], e01[:, 0:1], e01[:, 1:2])
                mf = smallp.tile([128, 1], f32, tag="mf")
                nc.scalar.activation(out=mf[:], in_=esum[:], func=Act.Ln,
                                     bias=1.0, scale=1.0)

                nc.vector.tensor_add(part_t[:, 0:1], part_t[:, 0:1], mf[:])
                nc.vector.tensor_add(part_t[:, 1:2], part_t[:, 1:2],
                                     sq_tot[:])

            if stage >= 2:
                # Pool stream: g00..g03, g10, sel(0,0), g11, sel(0,1),
                # g12, g13, sel(1,0), sel(1,1) — sel gathers slot between
                # later desc-gens so only sel(1,1) lands in the tail.
                sn_ts = [snp.tile([128, N_NEGS, HOPS], f32, tag="sn",
                                  name=f"sn{i}") for i in range(NCHUNK * K)]
                g00 = emit_gather(0, 0)
                g01 = emit_gather(0, 1)
                g02 = emit_gather(0, 2)
                g03 = emit_gather(0, 3)
                if stage >= 3:
                    score_quarter(0, 0, g00, sn_ts[0])
                    score_quarter(0, 1, g01, sn_ts[0])
                g10 = emit_gather(1, 0)
                if stage >= 4:
                    c4 = argmax_half(0, 0, sn_ts[0])
                    selr00 = sel_gather(0, 0, c4) if stage >= 5 else None
                if stage >= 3:
                    score_quarter(0, 2, g02, sn_ts[1])
                    score_quarter(0, 3, g03, sn_ts[1])
                g11 = emit_gather(1, 1)
                if stage >= 4:
                    c4 = argmax_half(0, 1, sn_ts[1])
                    selr01 = sel_gather(0, 1, c4) if stage >= 5 else None
                g12 = emit_gather(1, 2)
                g13 = emit_gather(1, 3)
                if stage >= 3:
                    score_quarter(1, 0, g10, sn_ts[2])
                    score_quarter(1, 1, g11, sn_ts[2])
                if stage >= 4:
                    c4 = argmax_half(1, 0, sn_ts[2])
                    selr10 = sel_gather(1, 0, c4) if stage >= 5 else None
                if stage >= 3:
                    score_quarter(1, 2, g12, sn_ts[3])
                    score_quarter(1, 3, g13, sn_ts[3])
                if stage >= 4:
                    c4 = argmax_half(1, 1, sn_ts[3])
                    selr11 = sel_gather(1, 1, c4) if stage >= 5 else None
                if stage >= 7:
                    loss_chunk(0, [selr00, selr01])
                    loss_chunk(1, [selr10, selr11])

            nc.sync.dma_start(part, part_t[:])
    nc.compile()
    return nc


def _build_runner(nc):
    import jax
    from jax.sharding import Mesh, PartitionSpec
    from jax.experimental.shard_map import shard_map
    from concourse import mybir
    from concourse.bass2jax import (install_neuronx_cc_hook,
                                    partition_id_tensor, _bass_exec_p)

    install_neuronx_cc_hook()
    partition_name = (nc.partition_id_tensor.name
                      if nc.partition_id_tensor else None)
    REPLICATED = set()

    in_names, out_names, out_avals, zero_outs = [], [], [], []
    for alloc in nc.m.functions[0].allocations:
        if not isinstance(alloc, mybir.MemoryLocationSet):
            continue
        name = alloc.memorylocations[0].name
        if alloc.kind == "ExternalInput":
            if name != partition_name:
                in_names.append(name)
        elif alloc.kind == "ExternalOutput":
            out_names.append(name)
            shape = tuple(alloc.tensor_shape)
            dtype = mybir.dt.np(alloc.dtype)
            out_avals.append(jax.core.ShapedArray(shape, dtype))
            zero_outs.append(np.zeros(shape, dtype))
    n_outs = len(out_avals)
    all_in_names = list(in_names) + list(out_names)
    if partition_name is not None:
        all_in_names.append(partition_name)

    def _body(*args):
        operands = list(args)
        if partition_name is not None:
            operands.append(partition_id_tensor())
        outs = _bass_exec_p.bind(
            *operands, out_avals=tuple(out_avals),
            in_names=tuple(all_in_names), out_names=tuple(out_names),
            lowering_input_output_aliases=(), sim_require_finite=True,
            sim_require_nnan=True, nc=nc)
        return tuple(outs)

    devices = jax.devices()[:NCORES]
    mesh = Mesh(np.asarray(devices), ("core",))
    spec_of = [
        PartitionSpec() if name in REPLICATED else PartitionSpec("core")
        for name in in_names
    ]
    in_specs = tuple(spec_of) + (PartitionSpec("core"),) * n_outs
    out_specs = (PartitionSpec("core"),) * n_outs
    sharded = jax.jit(
        shard_map(_body, mesh=mesh, in_specs=in_specs, out_specs=out_specs,
                  check_rep=False),
        keep_unused=True)
    shard_s = jax.sharding.NamedSharding(mesh, PartitionSpec("core"))
    repl_s = jax.sharding.NamedSharding(mesh, PartitionSpec())

    def run(per_core_maps, replicated_map):
        args = []
        for name in in_names:
            if name in REPLICATED:
                args.append(jax.device_put(replicated_map[name], repl_s))
            else:
                args.append(jax.device_put(
                    np.concatenate([m[name] for m in per_core_maps], axis=0),
                    shard_s))
        for z in zero_outs:
            args.append(jax.device_put(
                np.zeros((NCORES * z.shape[0], *z.shape[1:]), z.dtype),
                shard_s))
        outs = sharded(*args)
        return [
            {name: np.asarray(outs[i]).reshape(NCORES, *out_avals[i].shape)[c]
             for i, name in enumerate(out_names)}
            for c in range(NCORES)
        ]

    return run


def _get_runner():
    import os
    if "run" not in _CACHE:
        nc = _build_bass(int(os.environ.get("KSTAGE", "99")))
        _CACHE["nc"] = nc
        _CACHE["run"] = _build_runner(nc)
    return _CACHE["run"]


def _wrap_idx(arr):
    """dma_gather index layout: position i -> channel i%16, col i//16;
    the [16, n/16] block is replicated to all 128 partitions."""
    n = arr.shape[0]
    w = arr.reshape(n // 16, 16).T.astype(np.int16)
    return np.tile(w, (8, 1))


def make_in_maps(user_gcn_emb, item_gcn_emb, seed_embed, user, pos_item,
                 neg_item):
    """Host-side sharding/marshalling into per-core input maps."""
    user = np.asarray(user).astype(np.int64)
    pos_item = np.asarray(pos_item).astype(np.int64)
    neg_item = np.asarray(neg_item).astype(np.int64)
    seed = np.asarray(seed_embed, dtype=np.float32).reshape(BATCH)
    utab = np.ascontiguousarray(
        np.asarray(user_gcn_emb, dtype=np.float32).reshape(N_USERS, ROW))
    itab = np.ascontiguousarray(
        np.asarray(item_gcn_emb, dtype=np.float32).reshape(N_ITEMS, ROW))

    iota_block = np.tile(
        np.concatenate([
            (N_NEGS - np.arange(N_NEGS)).astype(np.float32),
            np.arange(HOPS, dtype=np.float32)]),
        (128, 1))

    per_core = []
    for c in range(NCORES):
        lo = c * B_LOC
        u = user[lo:lo + B_LOC]
        p = pos_item[lo:lo + B_LOC]
        nid = neg_item[lo:lo + B_LOC]                   # [256, 128]
        s_rows = utab[u].reshape(NCHUNK, 128, ROW).transpose(1, 0, 2).copy()
        p_rows = itab[p].reshape(NCHUNK, 128, ROW).transpose(1, 0, 2).copy()
        uniq, inv = np.unique(nid, return_inverse=True)
        rid = inv.reshape(B_LOC, NCAND)                 # values < len(uniq)
        subf = np.zeros((USUB, ROW), dtype=np.float32)
        subf[:len(uniq)] = itab[uniq]
        subb = subf.astype(ml_dtypes.bfloat16)
        cand = np.empty((128, NCHUNK * NQ, NIDX // 16), dtype=np.int16)
        for ch in range(NCHUNK):
            for q in range(NQ):
                arr = rid[ch * 128:(ch + 1) * 128,
                          q * SUB:(q + 1) * SUB].T.ravel()
                cand[:, ch * NQ + q, :] = _wrap_idx(arr)
        ridf = (rid.reshape(NCHUNK, 128, NCAND).transpose(1, 0, 2)
                .astype(np.float32))
        s = seed[lo:lo + B_LOC].reshape(NCHUNK, 128).T.copy()
        per_core.append({
            "subf": subf, "subb": subb, "s_rows": s_rows, "p_rows": p_rows,
            "cand16": cand, "ridf": ridf, "seed": s, "iotas": iota_block,
        })
    return per_core, {}


def combine(results):
    mf_sum = 0.0
    sq_sum = 0.0
    for r in results:
        mf_sum += float(r["part"][:, 0].astype(np.float64).sum())
        sq_sum += float(r["part"][:, 1].astype(np.float64).sum())
    mf_loss = np.float32(mf_sum / BATCH)
    emb_loss = np.float32(DECAY * sq_sum / 2.0 / BATCH)
    loss = np.float32(mf_loss + emb_loss)
    return loss, mf_loss, emb_loss


def kernel(user_gcn_emb, item_gcn_emb, seed_embed, user, pos_item, neg_item):
    run = _get_runner()
    per_core, replicated = make_in_maps(user_gcn_emb, item_gcn_emb,
                                        seed_embed, user, pos_item, neg_item)
    results = run(per_core, replicated)
    return combine(results)


# revision 26
# speedup vs baseline: 1.9527x; 1.2900x over previous
"""MixGCF negative-sampling + BPR loss kernel for 8x Trainium2 NeuronCores.

Strategy (data-parallel over batch, per-core row-sharded item subtables):
  - 8 cores x 256 users each (2 chunks of 128 users = partitions).
  - Host dedups each core's 32768 candidate item ids (np.unique -> at most
    32768 unique rows, so remapped ids always fit int16), ships per-core f32
    and bf16 subtables plus remapped indices; user/pos rows are pre-gathered
    densely on host (one row per batch element).
  - Device: per half-chunk (64 candidates x 128 users) one 8192-row
    dma_gather from the bf16 subtable; scores via bf16 multiply (DVE 2x
    mode) + an add-tree over the 64 dims (last levels in f32); per-hop
    argmax via max/iota-onehot; per-hop 64-float slices of the selected
    rows gathered from the f32 subtable; BPR loss in f32 reduced to
    per-partition partial sums. Host combines 8x[128,2] partials.
"""
import sys

sys.path.insert(0, "/opt/trn_rl_repo")
import numpy as np
import ml_dtypes

N_USERS = 200000
N_ITEMS = 200000
HOPS = 4
DIM = 64
BATCH = 2048
N_NEGS = 64
K = 2
DECAY = 1e-4
NCORES = 8
ROW = HOPS * DIM          # 256 f32 per table row
B_LOC = BATCH // NCORES   # 256 users per core
NCHUNK = B_LOC // 128     # 2 chunks of 128 users
NCAND = K * N_NEGS        # 128 candidates per user
NQ = 4                    # quarter-chunks per chunk (32 candidates each)
SUB = NCAND // NQ         # 32 candidates per quarter
NIDX = SUB * 128          # 4096 gathered rows per dma_gather
USUB = 32768              # per-core item subtable rows (padded)

_CACHE = {}


def _build_bass(stage=99):
    import concourse.bass as bass
    import concourse.tile as tile
    from concourse import bacc, mybir

    f32 = mybir.dt.float32
    bf16 = mybir.dt.bfloat16
    i32 = mybir.dt.int32
    i16 = mybir.dt.int16
    Alu = mybir.AluOpType
    Act = mybir.ActivationFunctionType

    nc = bacc.Bacc("TRN2", target_bir_lowering=False, debug=False,
                   num_devices=NCORES, dynamic_dma_scratch_size=32768,
                   num_swdge_queues=4)
    subf = nc.dram_tensor("subf", [USUB, ROW], f32, kind="ExternalInput").ap()
    subb = nc.dram_tensor("subb", [USUB, ROW], bf16,
                          kind="ExternalInput").ap()
    s_rows = nc.dram_tensor("s_rows", [128, NCHUNK, ROW], f32,
                            kind="ExternalInput").ap()
    p_rows = nc.dram_tensor("p_rows", [128, NCHUNK, ROW], f32,
                            kind="ExternalInput").ap()
    cand16 = nc.dram_tensor("cand16", [128, NCHUNK * NQ, NIDX // 16], i16,
                            kind="ExternalInput").ap()
    ridf = nc.dram_tensor("ridf", [128, NCHUNK, NCAND], f32,
                          kind="ExternalInput").ap()
    seed = nc.dram_tensor("seed", [128, NCHUNK], f32,
                          kind="ExternalInput").ap()
    iotas = nc.dram_tensor("iotas", [128, N_NEGS + HOPS], f32,
                           kind="ExternalInput").ap()
    part = nc.dram_tensor("part", [128, 2], f32, kind="ExternalOutput").ap()

    subf4 = subf.rearrange("u (h d) -> (u h) d", h=HOPS)

    with tile.TileContext(nc) as tc:
        with tc.tile_pool(name="meta", bufs=1) as meta, \
             tc.tile_pool(name="gat", bufs=4) as gatp, \
             tc.tile_pool(name="tree", bufs=1) as treep, \
             tc.tile_pool(name="sn", bufs=4) as snp, \
             tc.tile_pool(name="sel", bufs=4) as selp, \
             tc.tile_pool(name="small", bufs=2) as smallp:
            # ---- static/meta staging ----
            sp_t = meta.tile([128, NCHUNK, ROW], f32)
            pp_t = meta.tile([128, NCHUNK, ROW], f32)
            cand_t = meta.tile([128, NCHUNK * NQ, NIDX // 16], i16)
            ridf_t = meta.tile([128, NCHUNK, NCAND], f32)
            seed_t = meta.tile([128, NCHUNK], f32)
            iota_t = meta.tile([128, N_NEGS + HOPS], f32)
            nc.sync.dma_start(cand_t[:], cand16)
            nc.sync.dma_start(sp_t[:], s_rows)
            nc.sync.dma_start(pp_t[:], p_rows)
            nc.sync.dma_start(ridf_t[:], ridf)
            nc.sync.dma_start(seed_t[:], seed)
            nc.sync.dma_start(iota_t[:], iotas)
            iota_rev = iota_t[:, 0:N_NEGS]
            hpat = iota_t[:, N_NEGS:N_NEGS + HOPS]

            oms_t = meta.tile([128, NCHUNK], f32)   # 1 - seed
            nc.vector.tensor_scalar(oms_t[:], seed_t[:], -1.0, 1.0,
                                    Alu.mult, Alu.add)
            s_bf = meta.tile([128, NCHUNK, ROW], bf16)
            nc.vector.tensor_copy(s_bf[:], sp_t[:])

            part_t = meta.tile([128, 2], f32)
            nc.vector.memset(part_t[:], 0.0)

            # ---- hoisted loss prep (dense inputs only) ----
            u_sum = meta.tile([128, NCHUNK, DIM], f32)
            p_sum = meta.tile([128, NCHUNK, DIM], f32)
            psum_seed = meta.tile([128, NCHUNK, DIM], f32)
            for ch in range(NCHUNK):
                nc.vector.tensor_reduce(
                    out=u_sum[:, ch],
                    in_=sp_t[:, ch].rearrange("p (h d) -> p h d",
                                              h=HOPS).transpose([0, 2, 1]),
                    axis=mybir.AxisListType.X, op=Alu.add)
                nc.vector.tensor_reduce(
                    out=p_sum[:, ch],
                    in_=pp_t[:, ch].rearrange("p (h d) -> p h d",
                                              h=HOPS).transpose([0, 2, 1]),
                    axis=mybir.AxisListType.X, op=Alu.add)
                nc.vector.tensor_scalar_mul(psum_seed[:, ch], p_sum[:, ch],
                                            seed_t[:, ch:ch + 1])

            def emit_gather(ch, q):
                gat = gatp.tile([128, SUB, ROW], bf16, tag="gat",
                                name=f"gat{ch}{q}")
                nc.gpsimd.dma_gather(
                    out_ap=gat[:], in_ap=subb,
                    idxs_ap=cand_t[:, ch * NQ + q],
                    num_idxs=NIDX, num_idxs_reg=NIDX, elem_size=ROW,
                    single_packet=False, queue_num=(ch * NQ + q) % 4)
                return gat

            def score_quarter(ch, q, gat, sn_k):
                """bf16 mult by user row + add-tree over d -> sn_k slice."""
                qq = q % 2
                nc.vector.tensor_tensor(
                    out=gat[:], in0=gat[:],
                    in1=s_bf[:, ch].unsqueeze(1).to_broadcast(
                        [128, SUB, ROW]),
                    op=Alu.mult)
                v = gat[:].rearrange("p s (h d) -> p s h d", h=HOPS)
                t32 = treep.tile([128, SUB, HOPS, 32], bf16, tag="t32")
                nc.vector.tensor_tensor(out=t32[:], in0=v[:, :, :, 0:32],
                                        in1=v[:, :, :, 32:64], op=Alu.add)
                t16 = treep.tile([128, SUB, HOPS, 16], bf16, tag="t16")
                nc.vector.tensor_tensor(out=t16[:], in0=t32[:, :, :, 0:16],
                                        in1=t32[:, :, :, 16:32], op=Alu.add)
                t8 = treep.tile([128, SUB, HOPS, 8], bf16, tag="t8")
                nc.vector.tensor_tensor(out=t8[:], in0=t16[:, :, :, 0:8],
                                        in1=t16[:, :, :, 8:16], op=Alu.add)
                t4 = treep.tile([128, SUB, HOPS, 4], f32, tag="t4")
                nc.vector.tensor_tensor(out=t4[:], in0=t8[:, :, :, 0:4],
                                        in1=t8[:, :, :, 4:8], op=Alu.add)
                t2 = treep.tile([128, SUB, HOPS, 2], f32, tag="t2")
                nc.vector.tensor_tensor(out=t2[:], in0=t4[:, :, :, 0:2],
                                        in1=t4[:, :, :, 2:4], op=Alu.add)
                nc.vector.tensor_tensor(
                    out=sn_k[:, qq * SUB:(qq + 1) * SUB, :].unsqueeze(3),
                    in0=t2[:, :, :, 0:1], in1=t2[:, :, :, 1:2], op=Alu.add)

            def argmax_half(ch, k, sn_k):
                """per-hop argmax over this k-half -> 4*rid+h (i32)."""
                oms_ap = oms_t[:, ch:ch + 1]
                g_t = snp.tile([128, N_NEGS, HOPS], f32, tag="g")
                nc.vector.tensor_scalar_mul(g_t[:], sn_k[:], oms_ap)
                gk = g_t[:].transpose([0, 2, 1])       # [128, H, N]
                m_k = smallp.tile([128, HOPS], f32, tag="mk")
                nc.vector.tensor_reduce(out=m_k[:], in_=gk,
                                        axis=mybir.AxisListType.X,
                                        op=Alu.max)
                eq = smallp.tile([128, HOPS, N_NEGS], f32, tag="eq")
                nc.vector.tensor_tensor(
                    out=eq[:], in0=gk,
                    in1=m_k[:].unsqueeze(2).to_broadcast(
                        [128, HOPS, N_NEGS]),
                    op=Alu.is_equal)
                w = smallp.tile([128, HOPS, N_NEGS], f32, tag="w")
                nc.vector.tensor_tensor(
                    out=w[:], in0=eq[:],
                    in1=iota_rev.unsqueeze(1).to_broadcast(
                        [128, HOPS, N_NEGS]),
                    op=Alu.mult)
                wmax = smallp.tile([128, HOPS], f32, tag="wmax")
                nc.vector.tensor_reduce(out=wmax[:], in_=w[:],
                                        axis=mybir.AxisListType.X,
                                        op=Alu.max)
                onehot = smallp.tile([128, HOPS, N_NEGS], f32, tag="oh")
                nc.vector.tensor_tensor(
                    out=onehot[:],
                    in0=iota_rev.unsqueeze(1).to_broadcast(
                        [128, HOPS, N_NEGS]),
                    in1=wmax[:].unsqueeze(2).to_broadcast(
                        [128, HOPS, N_NEGS]),
                    op=Alu.is_equal)
                idsel = smallp.tile([128, HOPS, N_NEGS], f32, tag="ids")
                nc.vector.tensor_tensor(
                    out=idsel[:], in0=onehot[:],
                    in1=ridf_t[:, ch, k * N_NEGS:(k + 1) * N_NEGS]
                        .unsqueeze(1).to_broadcast([128, HOPS, N_NEGS]),
                    op=Alu.mult)
                candf = smallp.tile([128, HOPS], f32, tag="candf")
                nc.vector.tensor_reduce(out=candf[:], in_=idsel[:],
                                        axis=mybir.AxisListType.X,
                                        op=Alu.add)
                # idx into [USUB*HOPS, DIM] view: 4*rid + h
                idx4f = smallp.tile([128, HOPS], f32, tag="idx4f")
                nc.vector.tensor_scalar(idx4f[:], candf[:], float(HOPS), 0.0,
                                        Alu.mult, Alu.add)
                nc.vector.tensor_tensor(out=idx4f[:], in0=idx4f[:],
                                        in1=hpat, op=Alu.add)
                cand4_i = smallp.tile([128, HOPS], i32, tag="cand4i")
                nc.vector.tensor_copy(cand4_i[:], idx4f[:])
                return cand4_i

            def sel_gather(ch, k, cand4_i):
                selr = selp.tile([128, HOPS, DIM], f32, tag="selr",
                                 name=f"selr{ch}{k}")
                for j in range(HOPS):
                    nc.gpsimd.indirect_dma_start(
                        out=selr[:, j], out_offset=None,
                        in_=subf4,
                        in_offset=bass.IndirectOffsetOnAxis(
                            ap=cand4_i[:, j:j + 1], axis=0))
                return selr

            def loss_chunk(ch, selrs):
                seed_ap = seed_t[:, ch:ch + 1]
                oms_ap = oms_t[:, ch:ch + 1]
                n_sums = []
                for k in range(K):
                    selr = selrs[k]
                    r_k = smallp.tile([128, DIM], f32, tag=f"rk{k}")
                    nc.vector.tensor_add(r_k[:], selr[:, 0], selr[:, 1])
                    nc.vector.tensor_add(r_k[:], r_k[:], selr[:, 2])
                    nc.vector.tensor_add(r_k[:], r_k[:], selr[:, 3])
                    n_k = smallp.tile([128, DIM], f32, tag=f"nk{k}")
                    nc.vector.tensor_scalar_mul(n_k[:], r_k[:], oms_ap)
                    nc.vector.tensor_add(n_k[:], n_k[:], psum_seed[:, ch])
                    n_sums.append(n_k)

                # ---- loss pieces (scale 1/16 folds the /HOPS means) ----
                S = 1.0 / (HOPS * HOPS)
                tmp = smallp.tile([128, DIM], f32, tag="tmp")

                def dotp(out_ap, a, b):
                    nc.vector.tensor_tensor(out=tmp[:], in0=a, in1=b,
                                            op=Alu.mult)
                    nc.vector.tensor_reduce(out=out_ap, in_=tmp[:],
                                            axis=mybir.AxisListType.X,
                                            op=Alu.add)

                pos_s = smallp.tile([128, 1], f32, tag="poss")   # raw (x16)
                dotp(pos_s[:], u_sum[:, ch], p_sum[:, ch])
                neg_s = []
                for k in range(K):
                    ns = smallp.tile([128, 1], f32, tag=f"negs{k}")
                    dotp(ns[:], u_sum[:, ch], n_sums[k][:])
                    neg_s.append(ns)

                sq = smallp.tile([128, 4], f32, tag="sq")        # raw (x16)
                dotp(sq[:, 0:1], u_sum[:, ch], u_sum[:, ch])
                dotp(sq[:, 1:2], p_sum[:, ch], p_sum[:, ch])
                dotp(sq[:, 2:3], n_sums[0][:], n_sums[0][:])
                dotp(sq[:, 3:4], n_sums[1][:], n_sums[1][:])
                sq_tot = smallp.tile([128, 1], f32, tag="sqtot")
                nc.vector.tensor_add(sq_tot[:], sq[:, 0:1], sq[:, 1:2])
                nc.vector.tensor_add(sq_tot[:], sq_tot[:], sq[:, 2:3])
                nc.vector.tensor_add(sq_tot[:], sq_tot[:], sq[:, 3:4])
                nc.vector.tensor_scalar_mul(sq_tot[:], sq_tot[:], S)

                negpos = smallp.tile([128, 1], f32, tag="negpos")
                nc.vector.tensor_scalar_mul(negpos[:], pos_s[:], -S)
                e01 = smallp.tile([128, 2], f32, tag="e01")
                for k in range(K):
                    nc.scalar.activation(out=e01[:, k:k + 1],
                                         in_=neg_s[k][:], func=Act.Exp,
                                         bias=negpos[:], scale=S)
                esum = smallp.tile([128, 1], f32, tag="esum")
                nc.vector.tensor_add(esum[:], e01[:, 0:1], e01[:, 1:2])
                mf = smallp.tile([128, 1], f32, tag="mf")
                nc.scalar.activation(out=mf[:], in_=esum[:], func=Act.Ln,
                                     bias=1.0, scale=1.0)

                nc.vector.tensor_add(part_t[:, 0:1], part_t[:, 0:1], mf[:])
                nc.vector.tensor_add(part_t[:, 1:2], part_t[:, 1:2],
                                     sq_tot[:])

            if stage >= 2:
                # Pool stream: g00..g03, g10, sel(0,0), g11, sel(0,1),
                # g12, g13, sel(1,0), sel(1,1) — sel gathers slot between
                # later desc-gens so only sel(1,1) lands in the tail.
                sn_ts = [snp.tile([128, N_NEGS, HOPS], f32, tag="sn",
                                  name=f"sn{i}") for i in range(NCHUNK * K)]
                g00 = emit_gather(0, 0)
                g01 = emit_gather(0, 1)
                g02 = emit_gather(0, 2)
                g03 = emit_gather(0, 3)
                if stage >= 3:
                    score_quarter(0, 0, g00, sn_ts[0])
                    score_quarter(0, 1, g01, sn_ts[0])
                g10 = emit_gather(1, 0)
                if stage >= 4:
                    c4 = argmax_half(0, 0, sn_ts[0])
                    selr00 = sel_gather(0, 0, c4) if stage >= 5 else None
                if stage >= 3:
                    score_quarter(0, 2, g02, sn_ts[1])
                    score_quarter(0, 3, g03, sn_ts[1])
                g11 = emit_gather(1, 1)
                if stage >= 4:
                    c4 = argmax_half(0, 1, sn_ts[1])
                    selr01 = sel_gather(0, 1, c4) if stage >= 5 else None
                g12 = emit_gather(1, 2)
                g13 = emit_gather(1, 3)
                if stage >= 3:
                    score_quarter(1, 0, g10, sn_ts[2])
                    score_quarter(1, 1, g11, sn_ts[2])
                if stage >= 4:
                    c4 = argmax_half(1, 0, sn_ts[2])
                    selr10 = sel_gather(1, 0, c4) if stage >= 5 else None
                if stage >= 3:
                    score_quarter(1, 2, g12, sn_ts[3])
                    score_quarter(1, 3, g13, sn_ts[3])
                if stage >= 4:
                    c4 = argmax_half(1, 1, sn_ts[3])
                    selr11 = sel_gather(1, 1, c4) if stage >= 5 else None
                if stage >= 7:
                    loss_chunk(0, [selr00, selr01])
                    loss_chunk(1, [selr10, selr11])

            nc.sync.dma_start(part, part_t[:])
    nc.compile()
    return nc


def _build_runner(nc):
    import jax
    from jax.sharding import Mesh, PartitionSpec
    from jax.experimental.shard_map import shard_map
    from concourse import mybir
    from concourse.bass2jax import (install_neuronx_cc_hook,
                                    partition_id_tensor, _bass_exec_p)

    install_neuronx_cc_hook()
    partition_name = (nc.partition_id_tensor.name
                      if nc.partition_id_tensor else None)
    REPLICATED = set()

    in_names, out_names, out_avals, zero_outs = [], [], [], []
    for alloc in nc.m.functions[0].allocations:
        if not isinstance(alloc, mybir.MemoryLocationSet):
            continue
        name = alloc.memorylocations[0].name
        if alloc.kind == "ExternalInput":
            if name != partition_name:
                in_names.append(name)
        elif alloc.kind == "ExternalOutput":
            out_names.append(name)
            shape = tuple(alloc.tensor_shape)
            dtype = mybir.dt.np(alloc.dtype)
            out_avals.append(jax.core.ShapedArray(shape, dtype))
            zero_outs.append(np.zeros(shape, dtype))
    n_outs = len(out_avals)
    all_in_names = list(in_names) + list(out_names)
    if partition_name is not None:
        all_in_names.append(partition_name)

    def _body(*args):
        operands = list(args)
        if partition_name is not None:
            operands.append(partition_id_tensor())
        outs = _bass_exec_p.bind(
            *operands, out_avals=tuple(out_avals),
            in_names=tuple(all_in_names), out_names=tuple(out_names),
            lowering_input_output_aliases=(), sim_require_finite=True,
            sim_require_nnan=True, nc=nc)
        return tuple(outs)

    devices = jax.devices()[:NCORES]
    mesh = Mesh(np.asarray(devices), ("core",))
    spec_of = [
        PartitionSpec() if name in REPLICATED else PartitionSpec("core")
        for name in in_names
    ]
    in_specs = tuple(spec_of) + (PartitionSpec("core"),) * n_outs
    out_specs = (PartitionSpec("core"),) * n_outs
    sharded = jax.jit(
        shard_map(_body, mesh=mesh, in_specs=in_specs, out_specs=out_specs,
                  check_rep=False),
        keep_unused=True)
    shard_s = jax.sharding.NamedSharding(mesh, PartitionSpec("core"))
    repl_s = jax.sharding.NamedSharding(mesh, PartitionSpec())

    def run(per_core_maps, replicated_map):
        args = []
        for name in in_names:
            if name in REPLICATED:
                args.append(jax.device_put(replicated_map[name], repl_s))
            else:
                args.append(jax.device_put(
                    np.concatenate([m[name] for m in per_core_maps], axis=0),
                    shard_s))
        for z in zero_outs:
            args.append(jax.device_put(
                np.zeros((NCORES * z.shape[0], *z.shape[1:]), z.dtype),
                shard_s))
        outs = sharded(*args)
        return [
            {name: np.asarray(outs[i]).reshape(NCORES, *out_avals[i].shape)[c]
             for i, name in enumerate(out_names)}
            for c in range(NCORES)
        ]

    return run


def _get_runner():
    import os
    if "run" not in _CACHE:
        nc = _build_bass(int(os.environ.get("KSTAGE", "99")))
        _CACHE["nc"] = nc
        _CACHE["run"] = _build_runner(nc)
    return _CACHE["run"]


def _wrap_idx(arr):
    """dma_gather index layout: position i -> channel i%16, col i//16;
    the [16, n/16] block is replicated to all 128 partitions."""
    n = arr.shape[0]
    w = arr.reshape(n // 16, 16).T.astype(np.int16)
    return np.tile(w, (8, 1))


def make_in_maps(user_gcn_emb, item_gcn_emb, seed_embed, user, pos_item,
                 neg_item):
    """Host-side sharding/marshalling into per-core input maps."""
    user = np.asarray(user).astype(np.int64)
    pos_item = np.asarray(pos_item).astype(np.int64)
    neg_item = np.asarray(neg_item).astype(np.int64)
    seed = np.asarray(seed_embed, dtype=np.float32).reshape(BATCH)
    utab = np.ascontiguousarray(
        np.asarray(user_gcn_emb, dtype=np.float32).reshape(N_USERS, ROW))
    itab = np.ascontiguousarray(
        np.asarray(item_gcn_emb, dtype=np.float32).reshape(N_ITEMS, ROW))

    iota_block = np.tile(
        np.concatenate([
            (N_NEGS - np.arange(N_NEGS)).astype(np.float32),
            np.arange(HOPS, dtype=np.float32)]),
        (128, 1))

    per_core = []
    for c in range(NCORES):
        lo = c * B_LOC
        u = user[lo:lo + B_LOC]
        p = pos_item[lo:lo + B_LOC]
        nid = neg_item[lo:lo + B_LOC]                   # [256, 128]
        s_rows = utab[u].reshape(NCHUNK, 128, ROW).transpose(1, 0, 2).copy()
        p_rows = itab[p].reshape(NCHUNK, 128, ROW).transpose(1, 0, 2).copy()
        uniq, inv = np.unique(nid, return_inverse=True)
        rid = inv.reshape(B_LOC, NCAND)                 # values < len(uniq)
        subf = np.zeros((USUB, ROW), dtype=np.float32)
        subf[:len(uniq)] = itab[uniq]
        subb = subf.astype(ml_dtypes.bfloat16)
        cand = np.empty((128, NCHUNK * NQ, NIDX // 16), dtype=np.int16)
        for ch in range(NCHUNK):
            for q in range(NQ):
                arr = rid[ch * 128:(ch + 1) * 128,
                          q * SUB:(q + 1) * SUB].T.ravel()
                cand[:, ch * NQ + q, :] = _wrap_idx(arr)
        ridf = (rid.reshape(NCHUNK, 128, NCAND).transpose(1, 0, 2)
                .astype(np.float32))
        s = seed[lo:lo + B_LOC].reshape(NCHUNK, 128).T.copy()
        per_core.append({
            "subf": subf, "subb": subb, "s_rows": s_rows, "p_rows": p_rows,
            "cand16": cand, "ridf": ridf, "seed": s, "iotas": iota_block,
        })
    return per_core, {}


def combine(results):
    mf_sum = 0.0
    sq_sum = 0.0
    for r in results:
        mf_sum += float(r["part"][:, 0].astype(np.float64).sum())
        sq_sum += float(r["part"][:, 1].astype(np.float64).sum())
    mf_loss = np.float32(mf_sum / BATCH)
    emb_loss = np.float32(DECAY * sq_sum / 2.0 / BATCH)
    loss = np.float32(mf_loss + emb_loss)
    return loss, mf_loss, emb_loss


def kernel(user_gcn_emb, item_gcn_emb, seed_embed, user, pos_item, neg_item):
    run = _get_runner()
    per_core, replicated = make_in_maps(user_gcn_emb, item_gcn_emb,
                                        seed_embed, user, pos_item, neg_item)
    results = run(per_core, replicated)
    return combine(results)
